# revision 43
# baseline (speedup 1.0000x reference)
"""Bass/Tile TRN2 kernel for nn_CausalAttention (softmax + tril-matmul renorm).

Math restructuring (per core, row block of B = SEQ/n_cores rows):
    q = x @ wq ; k = x @ wk ; v = x @ wv
    z = q @ k.T / sqrt(D) ;  s = exp(z)              (softmax norm cancels below)
    masked[i,j] = sum_{t>=j} s[i,t]                  (suffix sum == s @ tril)
    out = (masked @ v) / rowsum(masked)

v2 identities (vs the tril/suffix formulation):
    masked @ v       == s @ cumsum(v)                 -> contract s with prefix-V
    rowsum(masked)   == s @ (t+1)                     -> one weight column
    z = q @ k.T      == ((x@wq) @ wk.T) @ x.T         -> gather RAW x, not K

so the AllGather input (x.T in fp8) is ready ~6us into the kernel instead of
after a full projection, and the O(S^2) tril matmuls + psum copies vanish.

Per-tile decomposition (tile r of 128 keys, T tiles):
    Vc[rP+j] = Vc0_r[j] + sum_{r'<r} VS[r']           (within-tile prefix + offsets)
    numT = sum_r Vc0_r.T-mm(st_r) + VS.T-mm(SUF)      SUF[r] = sum_{r'>r} CS[r']
    den[i] = sum_t (t+1) s[t,i]                       (selector-pair matmul w/ CS)

Layouts: everything transposed ([feature/key on partitions, query on free]).
Prefix-x trick: the x-transpose matmuls use rhs=[I | U] (U=upper-tri ones) so a
single pass yields both x.T (fp8, scores+gather) and prefix-x.T (bf16, V path);
xrs (tile row-sums of x) is the last U-column, read from PSUM in f32.

Collectives: cc1 = AllGather(x.T fp8) triggered right after the transposes;
cc2 = AllGather(Vc0 fp8 + xrs bf16) after the V projection. Both on the
otherwise-empty GpSimd queue so nothing delays the trigger.
"""
import numpy as np
from contextlib import ExitStack

import concourse.bass as bass
import concourse.tile as tile
from concourse import bacc, mybir

F32 = mybir.dt.float32
BF16 = mybir.dt.bfloat16
FP8 = mybir.dt.float8e4
U8 = mybir.dt.uint8
AX = mybir.AxisListType
AF = mybir.ActivationFunctionType
ALU = mybir.AluOpType

P = 128
MB = 48          # selector pair block half-width (padded for DoubleRow step%16)


def make_consts(T):
    iu = np.concatenate([np.eye(P, dtype=np.float32),
                         np.triu(np.ones((P, P), np.float32))], axis=1)
    stril = np.tri(T, T, -1, dtype=np.float32)  # [r', r] = 1 if r' > r
    # selector pair blocks (DoubleRow over tile pairs a=2pr, b=2pr+1):
    # ko0 col a / ko1 col b = 1 (per-tile colsum -> CS rows); col 32 = den
    # weight (t+1)/32, pinned to partition 32 for the later row extraction.
    NPAIR = T // 2
    selp = np.zeros((P, NPAIR * 2 * MB), np.float32)
    for pr in range(NPAIR):
        a, b = 2 * pr, 2 * pr + 1
        blk = pr * 2 * MB
        selp[:, blk + a] = 1.0
        selp[:, blk + 32] = (P * a + np.arange(P) + 1.0) / 32.0
        selp[:, blk + MB + b] = 1.0
        selp[:, blk + MB + 32] = (P * b + np.arange(P) + 1.0) / 32.0
    ident = np.eye(P, dtype=np.float32)
    import ml_dtypes
    bf = lambda a: a.astype(ml_dtypes.bfloat16)
    f8 = lambda a: a.astype(ml_dtypes.float8_e4m3)
    return dict(c_iu=bf(iu), c_stril=bf(stril), c_selp=f8(selp), c_ident=ident)


def build(SEQ=4096, D=1024, n_cores=8):
    T = SEQ // P           # global 128-key tiles
    TL = T // n_cores      # local tiles per core
    B = P * TL             # rows per core
    DC = D // P            # feature chunks
    W = min(512, D)        # moving free width for D-wide matmuls
    NH = D // W
    NPAIR = T // 2
    assert B <= 512 and T <= P and D % W == 0 and SEQ % (P * n_cores) == 0
    # wq prescaled x8, wk.T prescaled x2 -> z = 512 * z_true
    scale = float(1.0 / np.sqrt(D) / 16.0)
    EXPB = float(-np.log(16.0))   # st = s/16 keeps fp8e4 range safe

    nc = bacc.Bacc("TRN2", target_bir_lowering=False, debug=False, num_devices=n_cores)

    x = nc.dram_tensor("x", [B, D], BF16, kind="ExternalInput")
    wq_d = nc.dram_tensor("wq", [D, D], FP8, kind="ExternalInput")
    wkt_d = nc.dram_tensor("wk", [D, D], FP8, kind="ExternalInput")   # wk.T * 2
    wv_d = nc.dram_tensor("wv", [D, D], BF16, kind="ExternalInput")
    c_iu = nc.dram_tensor("c_iu", [P, 2 * P], BF16, kind="ExternalInput")
    c_stril = nc.dram_tensor("c_stril", [T, T], BF16, kind="ExternalInput")
    c_selp = nc.dram_tensor("c_selp", [P, NPAIR * 2 * MB], FP8, kind="ExternalInput")
    c_ident = nc.dram_tensor("c_ident", [P, P], F32, kind="ExternalInput")
    out = nc.dram_tensor("out", [B, D], F32, kind="ExternalOutput")

    # all four collectives are half-splits so the meshes pipeline with compute:
    # cc1a/cc1b = x.T key-halves (2 tiles each); cc2a/cc2b = Vc0 d-halves
    # (cc2a also carries xrs).
    TLH = TL // 2          # local tiles per key-half
    B2 = P * TLH
    KH = D * B2            # cc1{a,b}: xT8 half [P, DC*B2] fp8, flat (p k)
    D2 = D // 2
    VNB = B * D2           # cc2{a,b}: Vc0 d-half [P, TL*D2] fp8e4
    XRB = 2 * D * TL       # cc2a extra: xrs region [P, DC*TL] bf16

    with tile.TileContext(nc) as tc, ExitStack() as top:
        dram = top.enter_context(tc.tile_pool(name="dram", bufs=1, space="DRAM"))
        cc1a_in = dram.tile([KH], FP8)
        cc1a_out = dram.tile([n_cores, KH], FP8, addr_space="Shared")
        cc1b_in = dram.tile([KH], FP8)
        cc1b_out = dram.tile([n_cores, KH], FP8, addr_space="Shared")
        cc2a_in = dram.tile([VNB + XRB], U8)
        cc2a_out = dram.tile([n_cores, VNB + XRB], U8, addr_space="Shared")
        cc2b_in = dram.tile([VNB], U8)
        cc2b_out = dram.tile([n_cores, VNB], U8, addr_space="Shared")

        consts = top.enter_context(tc.tile_pool(name="consts", bufs=1))
        iu_sb = consts.tile([P, 2 * P], BF16)
        nc.scalar.dma_start(iu_sb[:], c_iu.ap())
        stril_sb = consts.tile([T, T], BF16)
        nc.scalar.dma_start(stril_sb[:], c_stril.ap())
        selp_sb = consts.tile([P, NPAIR * 2 * MB], FP8)
        nc.scalar.dma_start(selp_sb[:], c_selp.ap())
        ident_sb = consts.tile([P, P], F32)
        nc.scalar.dma_start(ident_sb[:], c_ident.ap())
        expb_sb = consts.tile([P, 1], F32)
        nc.vector.memset(expb_sb[:], EXPB)

        persist = top.enter_context(tc.tile_pool(name="persist", bufs=1))
        q2T = persist.tile([P, DC * B], FP8)         # (q @ wk.T).T row block
        st = persist.tile([P, T * B], FP8)           # exp(scores)/16, transposed
        vs_sb = persist.tile([T, D], BF16)           # per-tile V colsums
        xrs_s = persist.tile([P, n_cores * DC * TL], BF16)  # gathered x row sums [p,(c dc t)]
        xrs_g = persist.tile([P, DC * T], BF16)             # re-strided to [p,(dc r)]
        suf_sb = persist.tile([T, B], BF16)
        cs_sb = persist.tile([T, B], BF16)
        recip = persist.tile([P, TL], F32)
        dennat = persist.tile([P, TL], F32)
        den_pad = persist.tile([P, B], F32)
        wv_sb = persist.tile([P, DC * D], BF16)
        vpg = [[persist.tile([P, TL * D2], FP8, name=f"vp{g}_{rc}")
                for rc in range(n_cores)] for g in range(2)]

        # ------------- stage 1: transposes, gather-x, projections -------------
        with ExitStack() as s1:
            xpool = s1.enter_context(tc.tile_pool(name="xload", bufs=6))
            xTp = s1.enter_context(tc.tile_pool(name="xT", bufs=1))
            xT8 = xTp.tile([P, DC * B], FP8)     # x.T   (scores lhsT + cc1 input)
            xcT = xTp.tile([P, DC * B], BF16)    # prefix-x.T (V path)
            xrs_f = xTp.tile([P, DC * TL], F32)
            xrs_bf = xTp.tile([P, DC * TL], BF16)

            wpool = s1.enter_context(tc.tile_pool(name="w", bufs=1))
            wq_sb = wpool.tile([P, DC * D], FP8)
            wkt_sb = wpool.tile([P, DC * D], FP8)
            qT = wpool.tile([P, DC * B], FP8)

            trps = s1.enter_context(tc.tile_pool(name="trps", bufs=2, space="PSUM"))
            # x.T first (I pass) -> cc1 trigger ASAP; prefix-x.T (U pass) after
            xts = []
            for tcc in range(TL):
                xt_ = xpool.tile([P, D], BF16, tag="x", name=f"xt_{tcc}")
                (nc.sync if tcc < 2 else nc.scalar).dma_start(
                    xt_[:], x.ap()[tcc * P:(tcc + 1) * P, :])
                xts.append(xt_)
            cc1av = cc1a_in[0:KH].rearrange("(p k) -> p k", p=P)
            cc1bv = cc1b_in[0:KH].rearrange("(p k) -> p k", p=P)
            for dc in range(DC):
                psI = trps.tile([P, B], F32, tag="trI")
                for tcc in range(TL):
                    nc.tensor.matmul(psI[:, tcc * P:(tcc + 1) * P],
                                     xts[tcc][:, dc * P:(dc + 1) * P], iu_sb[:, 0:P],
                                     start=True, stop=True)
                (nc.vector.tensor_copy if dc % 2 == 0 else nc.scalar.copy)(
                    xT8[:, dc * B:(dc + 1) * B], psI[:])
                # stream x.T key-halves to the collective inputs as they land
                nc.sync.dma_start(cc1av[:, dc * B2:(dc + 1) * B2],
                                  xT8[:, dc * B:dc * B + B2])
                nc.sync.dma_start(cc1bv[:, dc * B2:(dc + 1) * B2],
                                  xT8[:, dc * B + B2:(dc + 1) * B])

            # gather x.T immediately (GpSimd queue is otherwise empty)
            nc.gpsimd.collective_compute(
                "AllGather", ALU.bypass,
                replica_groups=[list(range(n_cores))],
                ins=[cc1a_in.opt()], outs=[cc1a_out.opt()],
            )
            nc.gpsimd.collective_compute(
                "AllGather", ALU.bypass,
                replica_groups=[list(range(n_cores))],
                ins=[cc1b_in.opt()], outs=[cc1b_out.opt()],
            )

            for dc in range(DC):
                psU = trps.tile([P, B], F32, tag="trU")
                for tcc in range(TL):
                    nc.tensor.matmul(psU[:, tcc * P:(tcc + 1) * P],
                                     xts[tcc][:, dc * P:(dc + 1) * P], iu_sb[:, P:2 * P],
                                     start=True, stop=True)
                (nc.scalar.copy if dc % 2 == 0 else nc.vector.tensor_copy)(
                    xcT[:, dc * B:(dc + 1) * B], psU[:])
                nc.vector.tensor_copy(
                    xrs_f[:, dc * TL:(dc + 1) * TL]
                    .rearrange("p (t one) -> p t one", one=1),
                    psU.rearrange("p (t j) -> p t j", j=P)[:, :, P - 1:P])

            # weights (HWDGE queues; nothing gates the collective).
            # wv first: the V path gates cc2, which gates phase B.
            for dc in range(DC):
                nc.scalar.dma_start(wv_sb[:, dc * D:(dc + 1) * D], wv_d.ap()[dc * P:(dc + 1) * P, :])
            for dc in range(DC):
                nc.sync.dma_start(wq_sb[:, dc * D:(dc + 1) * D], wq_d.ap()[dc * P:(dc + 1) * P, :])
            for dc in range(DC):
                nc.scalar.dma_start(wkt_sb[:, dc * D:(dc + 1) * D], wkt_d.ap()[dc * P:(dc + 1) * P, :])

            pps = s1.enter_context(tc.tile_pool(name="pps", bufs=4, space="PSUM"))

            # Vc0 row blocks (within-tile prefix V, from prefix-x.T) -> cc2
            cc2av = cc2a_in[0:VNB].rearrange("(p k) -> p k", p=P)
            cc2bv = cc2b_in[0:VNB].rearrange("(p k) -> p k", p=P)
            for nh in range(NH):
                for tcc in range(TL):
                    vl = xpool.tile([P, D2], FP8, tag="vl")
                    v_ps = pps.tile([P, W], F32, tag="pp", name="v_ps")
                    for dci in range(DC):
                        nc.tensor.matmul(
                            v_ps[:],
                            xcT[:, dci * B + tcc * P: dci * B + (tcc + 1) * P],
                            wv_sb[:, dci * D + nh * W: dci * D + (nh + 1) * W],
                            start=(dci == 0), stop=(dci == DC - 1),
                        )
                    (nc.vector.tensor_copy if tcc % 2 == 0 else nc.scalar.copy)(
                        vl[:], v_ps[:])
                    nc.gpsimd.dma_start(
                        (cc2av if nh == 0 else cc2bv)[:, tcc * D2:(tcc + 1) * D2],
                        vl[:].bitcast(U8))
                if nh == 0:
                    # per-tile x row sums (f32 exact, from the U-part last col)
                    nc.vector.tensor_copy(xrs_bf[:], xrs_f[:])
                    cc2x = cc2a_in[VNB:VNB + XRB].rearrange("(p k) -> p k", p=P)
                    nc.gpsimd.dma_start(cc2x[:, :], xrs_bf[:].bitcast(U8))

            # qT = (x @ wq).T  then  q2T = (q @ wk.T).T   (both fp8 DoubleRow)
            wq3 = wq_sb.rearrange("p (dc d) -> p dc d", dc=DC)
            wkt3 = wkt_sb.rearrange("p (dc d) -> p dc d", dc=DC)
            xT83 = xT8.rearrange("p (dc b) -> p dc b", dc=DC)
            for dco in range(DC):
                q_ps = pps.tile([P, B], F32, tag="pp", name="q_ps")
                for pp_ in range(DC // 2):
                    nc.tensor.matmul(
                        q_ps[:],
                        wq3[:, 2 * pp_:2 * pp_ + 2, dco * P:(dco + 1) * P],
                        xT83[:, 2 * pp_:2 * pp_ + 2, :],
                        start=(pp_ == 0), stop=(pp_ == DC // 2 - 1),
                        perf_mode=mybir.MatmulPerfMode.DoubleRow,
                    )
                nc.vector.tensor_copy(qT[:, dco * B:(dco + 1) * B], q_ps[:])
            qT3 = qT.rearrange("p (dc b) -> p dc b", dc=DC)
            for dco in range(DC):
                q2_ps = pps.tile([P, B], F32, tag="pp", name="q2_ps")
                for pp_ in range(DC // 2):
                    nc.tensor.matmul(
                        q2_ps[:],
                        wkt3[:, 2 * pp_:2 * pp_ + 2, dco * P:(dco + 1) * P],
                        qT3[:, 2 * pp_:2 * pp_ + 2, :],
                        start=(pp_ == 0), stop=(pp_ == DC // 2 - 1),
                        perf_mode=mybir.MatmulPerfMode.DoubleRow,
                    )
                nc.vector.tensor_copy(q2T[:, dco * B:(dco + 1) * B], q2_ps[:])

        # second collective pair: gather Vc0 halves (+xrs with the first)
        nc.gpsimd.collective_compute(
            "AllGather", ALU.bypass,
            replica_groups=[list(range(n_cores))],
            ins=[cc2a_in.opt()], outs=[cc2a_out.opt()],
        )
        nc.gpsimd.collective_compute(
            "AllGather", ALU.bypass,
            replica_groups=[list(range(n_cores))],
            ins=[cc2b_in.opt()], outs=[cc2b_out.opt()],
        )
        # prefetch gathered Vc0 blocks (fires as soon as each gather lands)
        for rc in range(n_cores // 2):
            nc.gpsimd.dma_start(
                vpg[0][rc][:].bitcast(U8),
                cc2a_out[rc, 0:VNB].rearrange("(p k) -> p k", p=P))
        # gathered x row sums, per-core contiguous lines (128B per partition)
        for c in range(n_cores):
            nc.sync.dma_start(
                xrs_s[:, c * DC * TL:(c + 1) * DC * TL].bitcast(U8),
                cc2a_out[c, VNB:VNB + XRB].rearrange("(p k) -> p k", p=P))
        for rc in range(n_cores // 2):
            nc.gpsimd.dma_start(
                vpg[1][rc][:].bitcast(U8),
                cc2b_out[rc, 0:VNB].rearrange("(p k) -> p k", p=P))

        # ------------------- phase A: scores / exp / CS+den -------------------
        with ExitStack() as pa:
            ktp = pa.enter_context(tc.tile_pool(name="kt", bufs=1))
            ztp = pa.enter_context(tc.tile_pool(name="zt", bufs=3, space="PSUM"))
            csp = pa.enter_context(tc.tile_pool(name="csp", bufs=1, space="PSUM"))
            sfp = pa.enter_context(tc.tile_pool(name="sfp", bufs=2, space="PSUM"))
            cs_ps = csp.tile([33, B], F32)

            q2T3 = q2T.rearrange("p (dc b) -> p dc b", dc=DC)
            pi = 0
            for h in range(2):
                cch = cc1a_out if h == 0 else cc1b_out
                # preload the whole key-half: the loads race ahead of the
                # score matmuls, so no per-rc DMA wait gaps
                ktcs = []
                for rc in range(n_cores):
                    ktc = ktp.tile([P, DC * B2], FP8, tag="kt", name=f"ktc_{h}_{rc}")
                    nc.sync.dma_start(ktc[:], cch[rc, 0:KH].rearrange("(p k) -> p k", p=P))
                    ktcs.append(ktc)
                for rc in range(n_cores):
                    ktc3 = ktcs[rc].rearrange("p (dc i) -> p dc i", dc=DC)
                    for sub2 in range(TLH):
                        rg = rc * TL + h * TLH + sub2
                        zt = ztp.tile([P, B], F32, tag="zt")
                        for pp in range(DC // 2):
                            nc.tensor.matmul(
                                zt[:],
                                ktc3[:, 2 * pp:2 * pp + 2, sub2 * P:(sub2 + 1) * P],
                                q2T3[:, 2 * pp:2 * pp + 2, :],
                                start=(pp == 0), stop=(pp == DC // 2 - 1),
                                perf_mode=mybir.MatmulPerfMode.DoubleRow,
                            )
                        nc.scalar.activation(st[:, rg * B:(rg + 1) * B], zt[:],
                                             AF.Exp, bias=expb_sb[:], scale=scale)
                        if sub2 % 2 == 1:
                            pr = rg // 2
                            lp = (selp_sb[:, pr * 2 * MB:(pr + 1) * 2 * MB]
                                  .rearrange("p (two m) -> p two m", two=2)[:, :, 0:33])
                            rp = (st[:, (rg - 1) * B:(rg + 1) * B]
                                  .rearrange("p (two b) -> p two b", two=2))
                            nc.tensor.matmul(
                                cs_ps[:], lp, rp,
                                start=(pi == 0), stop=(pi == NPAIR - 1),
                                perf_mode=mybir.MatmulPerfMode.DoubleRow,
                            )
                            pi += 1

            for g in range(2):
                for rc in range(n_cores // 2, n_cores):
                    nc.sync.dma_start(
                        vpg[g][rc][:].bitcast(U8),
                        (cc2a_out if g == 0 else cc2b_out)[rc, 0:VNB]
                        .rearrange("(p k) -> p k", p=P))

            nc.vector.tensor_copy(cs_sb[:], cs_ps[0:T, :])
            nc.vector.memset(den_pad[:], 0.0)
            nc.vector.tensor_copy(den_pad[32:33, :], cs_ps[32:33, :])
            suf_ps = sfp.tile([T, B], F32)
            nc.tensor.matmul(suf_ps[:], stril_sb[:], cs_sb[:], start=True, stop=True)
            nc.scalar.copy(suf_sb[:], suf_ps[:])

            # 0.03125/den now, so the phase-B epilogues are never gated on it
            for sub in range(TL):
                dps = sfp.tile([P, P], F32, tag="dtp")
                nc.tensor.transpose(dps[:], den_pad[:, sub * P:(sub + 1) * P], ident_sb[:])
                nc.vector.tensor_scalar(dennat[:, sub:sub + 1], dps[:, 32:33], 32.0,
                                        None, op0=ALU.mult)
            nc.vector.reciprocal(recip[:], dennat[:])

        # ------------- phase B: num accumulation (natural layout) -------------
        # num[i, d] = sum_t st[t, i] Vc0[t, d] + sum_r SUF[r, i] VS[r, d];
        # st tile pairs are the stationary operand, Vc0 pairs the moving one,
        # so the output lands directly in [query, feature] layout: no
        # transposes, and the den scale is a per-partition scalar multiply.
        # VS is computed between the g0 sweep and its closes so the PE never
        # stalls on the xrs gather at the phase boundary.
        with ExitStack() as pb:
            outp = pb.enter_context(tc.tile_pool(name="outp", bufs=4))
            nump = pb.enter_context(tc.tile_pool(name="nump", bufs=4, space="PSUM"))
            vsps = pb.enter_context(tc.tile_pool(name="vsps", bufs=2, space="PSUM"))

            def sweep(g, nums):
                for rc in range(n_cores):
                    vp3 = vpg[g][rc].rearrange("p (t d) -> p t d", t=TL)
                    for pr in range(TL // 2):
                        rgb = (rc * TL + 2 * pr) * B
                        stp3 = (st[:, rgb: rgb + 2 * B]
                                .rearrange("p (two b) -> p two b", two=2))
                        rhs = vp3[:, 2 * pr:2 * pr + 2, :]
                        for ic in range(TL):
                            nc.tensor.matmul(
                                nums[ic][:],
                                stp3[:, :, ic * P:(ic + 1) * P],
                                rhs,
                                start=(rc == 0 and pr == 0), stop=False,
                                perf_mode=mybir.MatmulPerfMode.DoubleRow,
                            )

            def close_group(g, nums):
                for ic in range(TL):
                    nc.tensor.matmul(
                        nums[ic][:], suf_sb[:, ic * P:(ic + 1) * P],
                        vs_sb[:, g * D2:(g + 1) * D2],
                        start=False, stop=True,
                    )
                    ot = outp.tile([P, D2], F32, tag="ot", name=f"ot{g}_{ic}")
                    if ic % 2 == 0:
                        nc.vector.tensor_scalar(ot[:], nums[ic][:],
                                                recip[:, ic:ic + 1], None, op0=ALU.mult)
                    else:
                        nc.scalar.activation(ot[:], nums[ic][:], AF.Copy,
                                             scale=recip[:, ic:ic + 1])
                    (nc.sync if ic % 2 == 0 else nc.scalar).dma_start(
                        out.ap()[ic * P:(ic + 1) * P, g * D2:(g + 1) * D2], ot[:])

            nums0 = [nump.tile([P, D2], F32, tag="num", name=f"num_ps0_{ic}")
                     for ic in range(TL)]
            sweep(0, nums0)

            # VS = xrs.T-mm(wv)  [T, D] (gathered x row sums, re-strided)
            nc.vector.tensor_copy(
                xrs_g.rearrange("p (dc c t) -> p dc c t", dc=DC, c=n_cores),
                xrs_s.rearrange("p (c dc t) -> p dc c t", c=n_cores, dc=DC))
            for nh in range(NH):
                vs_ps = vsps.tile([T, W], F32, tag="vs")
                for dci in range(DC):
                    nc.tensor.matmul(
                        vs_ps[:],
                        xrs_g[:, dci * T:(dci + 1) * T],
                        wv_sb[:, dci * D + nh * W: dci * D + (nh + 1) * W],
                        start=(dci == 0), stop=(dci == DC - 1),
                    )
                nc.vector.tensor_copy(vs_sb[:, nh * W:(nh + 1) * W], vs_ps[:])

            close_group(0, nums0)
            nums1 = [nump.tile([P, D2], F32, tag="num", name=f"num_ps1_{ic}")
                     for ic in range(TL)]
            sweep(1, nums1)
            close_group(1, nums1)

    nc.compile()
    return nc


def make_in_maps(x_full, wq, wk, wv, n_cores=8):
    import ml_dtypes
    bf = lambda a: np.ascontiguousarray(a).astype(ml_dtypes.bfloat16)
    f8 = lambda a: np.ascontiguousarray(a).astype(ml_dtypes.float8_e4m3)
    SEQ, D = x_full.shape
    T = SEQ // P
    B = SEQ // n_cores
    consts = make_consts(T)
    wq8 = f8(wq * 8.0)
    wkt2 = f8(wk.T * 2.0)
    wvb = bf(wv)
    in_maps = []
    for c in range(n_cores):
        m = {"x": bf(x_full[c * B:(c + 1) * B]),
             "wq": wq8, "wk": wkt2, "wv": wvb}
        m.update(consts)
        in_maps.append(m)
    return in_maps


def algo_ref(x, wq, wk, wv):
    """Numpy float64 reference of the restructured math (for validation)."""
    x = x.astype(np.float64)
    q2 = (x @ wq.astype(np.float64)) @ wk.astype(np.float64).T
    s = np.exp(q2 @ x.T / np.sqrt(x.shape[1]))
    Vc = np.cumsum(x @ wv.astype(np.float64), axis=0)
    num = s @ Vc
    den = s @ (np.arange(x.shape[0]) + 1.0)
    return (num / den[:, None]).astype(np.float32)


# ----------------------------------------------------------------------------
# Harness entry point: full (unsharded) inputs -> full output.
# ----------------------------------------------------------------------------
SEQ, D_IN, N_CORES = 4096, 1024, 8
_built = {}


def _get_nc(SEQ_=SEQ, D_=D_IN, n_cores=N_CORES):
    key = (SEQ_, D_, n_cores)
    if key not in _built:
        _built[key] = build(SEQ=SEQ_, D=D_, n_cores=n_cores)
    return _built[key]


def run(x, wq, wk, wv, trace=False, **spmd_kwargs):
    from concourse.bass_utils import run_bass_kernel_spmd

    x = np.ascontiguousarray(np.asarray(x, dtype=np.float32))
    wq = np.ascontiguousarray(np.asarray(wq, dtype=np.float32))
    wk = np.ascontiguousarray(np.asarray(wk, dtype=np.float32))
    wv = np.ascontiguousarray(np.asarray(wv, dtype=np.float32))
    n_cores = N_CORES
    nc = _get_nc(x.shape[0], x.shape[1], n_cores)
    in_maps = make_in_maps(x, wq, wk, wv, n_cores=n_cores)
    res = run_bass_kernel_spmd(nc, in_maps, list(range(n_cores)),
                               trace=trace, **spmd_kwargs)
    out = np.concatenate([res.results[c]["out"] for c in range(n_cores)], axis=0)
    return out, res


def kernel(x, wq, wk, wv):
    out, _ = run(x, wq, wk, wv, trace=False)
    return out


# revision 45
# speedup vs baseline: 1.2819x; 1.2819x over previous
"""Bass/Tile TRN2 kernel for nn_CausalAttention (softmax + tril-matmul renorm).

Math restructuring (per core, row block of B = SEQ/n_cores rows):
    q = x @ wq ; k = x @ wk ; v = x @ wv
    z = q @ k.T / sqrt(D) ;  s = exp(z)              (softmax norm cancels below)
    masked[i,j] = sum_{t>=j} s[i,t]                  (suffix sum == s @ tril)
    out = (masked @ v) / rowsum(masked)

v2 identities (vs the tril/suffix formulation):
    masked @ v       == s @ cumsum(v)                 -> contract s with prefix-V
    rowsum(masked)   == s @ (t+1)                     -> one weight column
    z = q @ k.T      == ((x@wq) @ wk.T) @ x.T         -> gather RAW x, not K

so the AllGather input (x.T in fp8) is ready ~6us into the kernel instead of
after a full projection, and the O(S^2) tril matmuls + psum copies vanish.

Per-tile decomposition (tile r of 128 keys, T tiles):
    Vc[rP+j] = Vc0_r[j] + sum_{r'<r} VS[r']           (within-tile prefix + offsets)
    numT = sum_r Vc0_r.T-mm(st_r) + VS.T-mm(SUF)      SUF[r] = sum_{r'>r} CS[r']
    den[i] = sum_t (t+1) s[t,i]                       (selector-pair matmul w/ CS)

Layouts: everything transposed ([feature/key on partitions, query on free]).
Prefix-x trick: the x-transpose matmuls use rhs=[I | U] (U=upper-tri ones) so a
single pass yields both x.T (fp8, scores+gather) and prefix-x.T (bf16, V path);
xrs (tile row-sums of x) is the last U-column, read from PSUM in f32.

Collectives: cc1 = AllGather(x.T fp8) triggered right after the transposes;
cc2 = AllGather(Vc0 fp8 + xrs bf16) after the V projection. Both on the
otherwise-empty GpSimd queue so nothing delays the trigger.
"""
import numpy as np
from contextlib import ExitStack

import concourse.bass as bass
import concourse.tile as tile
from concourse import bacc, mybir

F32 = mybir.dt.float32
BF16 = mybir.dt.bfloat16
FP8 = mybir.dt.float8e4
U8 = mybir.dt.uint8
AX = mybir.AxisListType
AF = mybir.ActivationFunctionType
ALU = mybir.AluOpType

P = 128
MB = 48          # selector pair block half-width (padded for DoubleRow step%16)


def make_consts(T):
    iu = np.concatenate([np.eye(P, dtype=np.float32),
                         np.triu(np.ones((P, P), np.float32))], axis=1)
    stril = np.tri(T, T, -1, dtype=np.float32)  # [r', r] = 1 if r' > r
    # selector pair blocks (DoubleRow over tile pairs a=2pr, b=2pr+1):
    # ko0 col a / ko1 col b = 1 (per-tile colsum -> CS rows); col 32 = den
    # weight (t+1)/32, pinned to partition 32 for the later row extraction.
    NPAIR = T // 2
    selp = np.zeros((P, NPAIR * 2 * MB), np.float32)
    for pr in range(NPAIR):
        a, b = 2 * pr, 2 * pr + 1
        blk = pr * 2 * MB
        selp[:, blk + a] = 1.0
        selp[:, blk + 32] = (P * a + np.arange(P) + 1.0) / 32.0
        selp[:, blk + MB + b] = 1.0
        selp[:, blk + MB + 32] = (P * b + np.arange(P) + 1.0) / 32.0
    ident = np.eye(P, dtype=np.float32)
    import ml_dtypes
    bf = lambda a: a.astype(ml_dtypes.bfloat16)
    f8 = lambda a: a.astype(ml_dtypes.float8_e4m3)
    return dict(c_iu=bf(iu), c_stril=bf(stril), c_selp=f8(selp), c_ident=ident)


def build(SEQ=4096, D=1024, n_cores=8):
    T = SEQ // P           # global 128-key tiles
    TL = T // n_cores      # local tiles per core
    B = P * TL             # rows per core
    DC = D // P            # feature chunks
    W = min(512, D)        # moving free width for D-wide matmuls
    NH = D // W
    NPAIR = T // 2
    assert B <= 512 and T <= P and D % W == 0 and SEQ % (P * n_cores) == 0
    # wq prescaled x8, wk.T prescaled x2 -> z = 512 * z_true
    scale = float(1.0 / np.sqrt(D) / 16.0)
    EXPB = float(-np.log(16.0))   # st = s/16 keeps fp8e4 range safe

    nc = bacc.Bacc("TRN2", target_bir_lowering=False, debug=False, num_devices=n_cores)

    x = nc.dram_tensor("x", [B, D], BF16, kind="ExternalInput")
    wq_d = nc.dram_tensor("wq", [D, D], FP8, kind="ExternalInput")
    wkt_d = nc.dram_tensor("wk", [D, D], FP8, kind="ExternalInput")   # wk.T * 2
    wv_d = nc.dram_tensor("wv", [D, D], BF16, kind="ExternalInput")
    c_iu = nc.dram_tensor("c_iu", [P, 2 * P], BF16, kind="ExternalInput")
    c_stril = nc.dram_tensor("c_stril", [T, T], BF16, kind="ExternalInput")
    c_selp = nc.dram_tensor("c_selp", [P, NPAIR * 2 * MB], FP8, kind="ExternalInput")
    c_ident = nc.dram_tensor("c_ident", [P, P], F32, kind="ExternalInput")
    out = nc.dram_tensor("out", [B, D], F32, kind="ExternalOutput")

    # all four collectives are half-splits so the meshes pipeline with compute:
    # cc1a/cc1b = x.T key-halves (2 tiles each); cc2a/cc2b = Vc0 d-halves
    # (cc2a also carries xrs).
    TLH = TL // 2          # local tiles per key-half
    B2 = P * TLH
    KH = D * B2            # cc1{a,b}: xT8 half [P, DC*B2] fp8, flat (p k)
    D2 = D // 2
    VNB = B * D2           # cc2{a,b}: Vc0 d-half [P, TL*D2] fp8e4
    XRB = 2 * D * TL       # cc2a extra: xrs region [P, DC*TL] bf16

    with tile.TileContext(nc) as tc, ExitStack() as top:
        dram = top.enter_context(tc.tile_pool(name="dram", bufs=1, space="DRAM"))
        cc1a_in = dram.tile([KH], FP8)
        cc1a_out = dram.tile([n_cores, KH], FP8, addr_space="Shared")
        cc1b_in = dram.tile([KH], FP8)
        cc1b_out = dram.tile([n_cores, KH], FP8, addr_space="Shared")
        cc2a_in = dram.tile([VNB + XRB], U8)
        cc2a_out = dram.tile([n_cores, VNB + XRB], U8, addr_space="Shared")
        cc2b_in = dram.tile([VNB], U8)
        cc2b_out = dram.tile([n_cores, VNB], U8, addr_space="Shared")

        consts = top.enter_context(tc.tile_pool(name="consts", bufs=1))
        iu_sb = consts.tile([P, 2 * P], BF16)
        nc.scalar.dma_start(iu_sb[:], c_iu.ap())
        stril_sb = consts.tile([T, T], BF16)
        nc.scalar.dma_start(stril_sb[:], c_stril.ap())
        selp_sb = consts.tile([P, NPAIR * 2 * MB], FP8)
        nc.scalar.dma_start(selp_sb[:], c_selp.ap())
        ident_sb = consts.tile([P, P], F32)
        nc.scalar.dma_start(ident_sb[:], c_ident.ap())
        expb_sb = consts.tile([P, 1], F32)
        nc.vector.memset(expb_sb[:], EXPB)

        persist = top.enter_context(tc.tile_pool(name="persist", bufs=1))
        q2T = persist.tile([P, DC * B], FP8)         # (q @ wk.T).T row block
        st = persist.tile([P, T * B], FP8)           # exp(scores)/16, transposed
        vs_sb = persist.tile([T, D], BF16)           # per-tile V colsums
        xrs_s = persist.tile([P, n_cores * DC * TL], BF16)  # gathered x row sums [p,(c dc t)]
        xrs_g = persist.tile([P, DC * T], BF16)             # re-strided to [p,(dc r)]
        suf_sb = persist.tile([T, B], BF16)
        cs_sb = persist.tile([T, B], BF16)
        recip = persist.tile([P, TL], F32)
        dennat = persist.tile([P, TL], F32)
        den_pad = persist.tile([P, B], F32)
        wv_sb = persist.tile([P, DC * D], BF16)
        vpg = [[persist.tile([P, TL * D2], FP8, name=f"vp{g}_{rc}")
                for rc in range(n_cores)] for g in range(2)]

        # ------------- stage 1: transposes, gather-x, projections -------------
        with ExitStack() as s1:
            xpool = s1.enter_context(tc.tile_pool(name="xload", bufs=6))
            xTp = s1.enter_context(tc.tile_pool(name="xT", bufs=1))
            xT8 = xTp.tile([P, DC * B], FP8)     # x.T   (scores lhsT + cc1 input)
            xcT = xTp.tile([P, DC * B], BF16)    # prefix-x.T (V path)
            xrs_f = xTp.tile([P, DC * TL], F32)
            xrs_bf = xTp.tile([P, DC * TL], BF16)

            wpool = s1.enter_context(tc.tile_pool(name="w", bufs=1))
            wq_sb = wpool.tile([P, DC * D], FP8)
            wkt_sb = wpool.tile([P, DC * D], FP8)
            qT = wpool.tile([P, DC * B], FP8)

            trps = s1.enter_context(tc.tile_pool(name="trps", bufs=2, space="PSUM"))
            # x.T first (I pass) -> cc1 trigger ASAP; prefix-x.T (U pass) after
            xts = []
            for tcc in range(TL):
                xt_ = xpool.tile([P, D], BF16, tag="x", name=f"xt_{tcc}")
                (nc.sync if tcc < 2 else nc.scalar).dma_start(
                    xt_[:], x.ap()[tcc * P:(tcc + 1) * P, :])
                xts.append(xt_)
            cc1av = cc1a_in[0:KH].rearrange("(p k) -> p k", p=P)
            cc1bv = cc1b_in[0:KH].rearrange("(p k) -> p k", p=P)
            for dc in range(DC):
                psI = trps.tile([P, B], F32, tag="trI")
                for tcc in range(TL):
                    nc.tensor.matmul(psI[:, tcc * P:(tcc + 1) * P],
                                     xts[tcc][:, dc * P:(dc + 1) * P], iu_sb[:, 0:P],
                                     start=True, stop=True)
                (nc.vector.tensor_copy if dc % 2 == 0 else nc.scalar.copy)(
                    xT8[:, dc * B:(dc + 1) * B], psI[:])
                # stream x.T key-halves to the collective inputs as they land
                nc.sync.dma_start(cc1av[:, dc * B2:(dc + 1) * B2],
                                  xT8[:, dc * B:dc * B + B2])
                nc.sync.dma_start(cc1bv[:, dc * B2:(dc + 1) * B2],
                                  xT8[:, dc * B + B2:(dc + 1) * B])

            # gather x.T immediately (GpSimd queue is otherwise empty)
            nc.gpsimd.collective_compute(
                "AllGather", ALU.bypass,
                replica_groups=[list(range(n_cores))],
                ins=[cc1a_in.opt()], outs=[cc1a_out.opt()],
            )
            nc.gpsimd.collective_compute(
                "AllGather", ALU.bypass,
                replica_groups=[list(range(n_cores))],
                ins=[cc1b_in.opt()], outs=[cc1b_out.opt()],
            )

            for dc in range(DC):
                psU = trps.tile([P, B], F32, tag="trU")
                for tcc in range(TL):
                    nc.tensor.matmul(psU[:, tcc * P:(tcc + 1) * P],
                                     xts[tcc][:, dc * P:(dc + 1) * P], iu_sb[:, P:2 * P],
                                     start=True, stop=True)
                (nc.scalar.copy if dc % 2 == 0 else nc.vector.tensor_copy)(
                    xcT[:, dc * B:(dc + 1) * B], psU[:])
                nc.vector.tensor_copy(
                    xrs_f[:, dc * TL:(dc + 1) * TL]
                    .rearrange("p (t one) -> p t one", one=1),
                    psU.rearrange("p (t j) -> p t j", j=P)[:, :, P - 1:P])

            # weights (HWDGE queues; nothing gates the collective).
            # wv first: the V path gates cc2, which gates phase B.
            for dc in range(DC):
                nc.scalar.dma_start(wv_sb[:, dc * D:(dc + 1) * D], wv_d.ap()[dc * P:(dc + 1) * P, :])
            for dc in range(DC):
                nc.sync.dma_start(wq_sb[:, dc * D:(dc + 1) * D], wq_d.ap()[dc * P:(dc + 1) * P, :])
            for dc in range(DC):
                nc.scalar.dma_start(wkt_sb[:, dc * D:(dc + 1) * D], wkt_d.ap()[dc * P:(dc + 1) * P, :])

            pps = s1.enter_context(tc.tile_pool(name="pps", bufs=4, space="PSUM"))

            # Vc0 row blocks (within-tile prefix V, from prefix-x.T) -> cc2
            cc2av = cc2a_in[0:VNB].rearrange("(p k) -> p k", p=P)
            cc2bv = cc2b_in[0:VNB].rearrange("(p k) -> p k", p=P)
            for nh in range(NH):
                for tcc in range(TL):
                    vl = xpool.tile([P, D2], FP8, tag="vl")
                    v_ps = pps.tile([P, W], F32, tag="pp", name="v_ps")
                    for dci in range(DC):
                        nc.tensor.matmul(
                            v_ps[:],
                            xcT[:, dci * B + tcc * P: dci * B + (tcc + 1) * P],
                            wv_sb[:, dci * D + nh * W: dci * D + (nh + 1) * W],
                            start=(dci == 0), stop=(dci == DC - 1),
                        )
                    (nc.vector.tensor_copy if tcc % 2 == 0 else nc.scalar.copy)(
                        vl[:], v_ps[:])
                    nc.gpsimd.dma_start(
                        (cc2av if nh == 0 else cc2bv)[:, tcc * D2:(tcc + 1) * D2],
                        vl[:].bitcast(U8))
                if nh == 0:
                    # per-tile x row sums (f32 exact, from the U-part last col)
                    nc.vector.tensor_copy(xrs_bf[:], xrs_f[:])
                    cc2x = cc2a_in[VNB:VNB + XRB].rearrange("(p k) -> p k", p=P)
                    nc.gpsimd.dma_start(cc2x[:, :], xrs_bf[:].bitcast(U8))

            # qT = (x @ wq).T  then  q2T = (q @ wk.T).T   (both fp8 DoubleRow)
            wq3 = wq_sb.rearrange("p (dc d) -> p dc d", dc=DC)
            wkt3 = wkt_sb.rearrange("p (dc d) -> p dc d", dc=DC)
            xT83 = xT8.rearrange("p (dc b) -> p dc b", dc=DC)
            for dco in range(DC):
                q_ps = pps.tile([P, B], F32, tag="pp", name="q_ps")
                for pp_ in range(DC // 2):
                    nc.tensor.matmul(
                        q_ps[:],
                        wq3[:, 2 * pp_:2 * pp_ + 2, dco * P:(dco + 1) * P],
                        xT83[:, 2 * pp_:2 * pp_ + 2, :],
                        start=(pp_ == 0), stop=(pp_ == DC // 2 - 1),
                        perf_mode=mybir.MatmulPerfMode.DoubleRow,
                    )
                nc.vector.tensor_copy(qT[:, dco * B:(dco + 1) * B], q_ps[:])
            qT3 = qT.rearrange("p (dc b) -> p dc b", dc=DC)
            for dco in range(DC):
                q2_ps = pps.tile([P, B], F32, tag="pp", name="q2_ps")
                for pp_ in range(DC // 2):
                    nc.tensor.matmul(
                        q2_ps[:],
                        wkt3[:, 2 * pp_:2 * pp_ + 2, dco * P:(dco + 1) * P],
                        qT3[:, 2 * pp_:2 * pp_ + 2, :],
                        start=(pp_ == 0), stop=(pp_ == DC // 2 - 1),
                        perf_mode=mybir.MatmulPerfMode.DoubleRow,
                    )
                nc.vector.tensor_copy(q2T[:, dco * B:(dco + 1) * B], q2_ps[:])

        # second collective pair: gather Vc0 halves (+xrs with the first)
        nc.gpsimd.collective_compute(
            "AllGather", ALU.bypass,
            replica_groups=[list(range(n_cores))],
            ins=[cc2a_in.opt()], outs=[cc2a_out.opt()],
        )
        nc.gpsimd.collective_compute(
            "AllGather", ALU.bypass,
            replica_groups=[list(range(n_cores))],
            ins=[cc2b_in.opt()], outs=[cc2b_out.opt()],
        )
        # prefetch gathered Vc0 blocks (fires as soon as each gather lands)
        for rc in range(n_cores // 2):
            nc.gpsimd.dma_start(
                vpg[0][rc][:].bitcast(U8),
                cc2a_out[rc, 0:VNB].rearrange("(p k) -> p k", p=P))
        # gathered x row sums, per-core contiguous lines (128B per partition)
        for c in range(n_cores):
            nc.sync.dma_start(
                xrs_s[:, c * DC * TL:(c + 1) * DC * TL].bitcast(U8),
                cc2a_out[c, VNB:VNB + XRB].rearrange("(p k) -> p k", p=P))
        for rc in range(n_cores // 2):
            nc.gpsimd.dma_start(
                vpg[1][rc][:].bitcast(U8),
                cc2b_out[rc, 0:VNB].rearrange("(p k) -> p k", p=P))

        # ------------------- phase A: scores / exp / CS+den -------------------
        with ExitStack() as pa:
            ktp = pa.enter_context(tc.tile_pool(name="kt", bufs=3))
            ztp = pa.enter_context(tc.tile_pool(name="zt", bufs=3, space="PSUM"))
            csp = pa.enter_context(tc.tile_pool(name="csp", bufs=1, space="PSUM"))
            sfp = pa.enter_context(tc.tile_pool(name="sfp", bufs=2, space="PSUM"))
            cs_ps = csp.tile([33, B], F32)

            q2T3 = q2T.rearrange("p (dc b) -> p dc b", dc=DC)
            pi = 0
            for h in range(2):
                cch = cc1a_out if h == 0 else cc1b_out
                for rc in range(n_cores):
                    ktc = ktp.tile([P, DC * B2], FP8, tag="kt")
                    nc.sync.dma_start(ktc[:], cch[rc, 0:KH].rearrange("(p k) -> p k", p=P))
                    ktc3 = ktc.rearrange("p (dc i) -> p dc i", dc=DC)
                    for sub2 in range(TLH):
                        rg = rc * TL + h * TLH + sub2
                        zt = ztp.tile([P, B], F32, tag="zt")
                        for pp in range(DC // 2):
                            nc.tensor.matmul(
                                zt[:],
                                ktc3[:, 2 * pp:2 * pp + 2, sub2 * P:(sub2 + 1) * P],
                                q2T3[:, 2 * pp:2 * pp + 2, :],
                                start=(pp == 0), stop=(pp == DC // 2 - 1),
                                perf_mode=mybir.MatmulPerfMode.DoubleRow,
                            )
                        nc.scalar.activation(st[:, rg * B:(rg + 1) * B], zt[:],
                                             AF.Exp, bias=expb_sb[:], scale=scale)
                        if sub2 % 2 == 1:
                            pr = rg // 2
                            lp = (selp_sb[:, pr * 2 * MB:(pr + 1) * 2 * MB]
                                  .rearrange("p (two m) -> p two m", two=2)[:, :, 0:33])
                            rp = (st[:, (rg - 1) * B:(rg + 1) * B]
                                  .rearrange("p (two b) -> p two b", two=2))
                            nc.tensor.matmul(
                                cs_ps[:], lp, rp,
                                start=(pi == 0), stop=(pi == NPAIR - 1),
                                perf_mode=mybir.MatmulPerfMode.DoubleRow,
                            )
                            pi += 1

            for g in range(2):
                for rc in range(n_cores // 2, n_cores):
                    nc.sync.dma_start(
                        vpg[g][rc][:].bitcast(U8),
                        (cc2a_out if g == 0 else cc2b_out)[rc, 0:VNB]
                        .rearrange("(p k) -> p k", p=P))

            nc.vector.tensor_copy(cs_sb[:], cs_ps[0:T, :])
            nc.vector.memset(den_pad[:], 0.0)
            nc.vector.tensor_copy(den_pad[32:33, :], cs_ps[32:33, :])
            suf_ps = sfp.tile([T, B], F32)
            nc.tensor.matmul(suf_ps[:], stril_sb[:], cs_sb[:], start=True, stop=True)
            nc.scalar.copy(suf_sb[:], suf_ps[:])

            # 0.03125/den now, so the phase-B epilogues are never gated on it
            for sub in range(TL):
                dps = sfp.tile([P, P], F32, tag="dtp")
                nc.tensor.transpose(dps[:], den_pad[:, sub * P:(sub + 1) * P], ident_sb[:])
                nc.vector.tensor_scalar(dennat[:, sub:sub + 1], dps[:, 32:33], 32.0,
                                        None, op0=ALU.mult)
            nc.vector.reciprocal(recip[:], dennat[:])

        # ------------- phase B: num accumulation (natural layout) -------------
        # num[i, d] = sum_t st[t, i] Vc0[t, d] + sum_r SUF[r, i] VS[r, d];
        # st tile pairs are the stationary operand, Vc0 pairs the moving one,
        # so the output lands directly in [query, feature] layout: no
        # transposes, and the den scale is a per-partition scalar multiply.
        # VS is computed between the g0 sweep and its closes so the PE never
        # stalls on the xrs gather at the phase boundary.
        with ExitStack() as pb:
            outp = pb.enter_context(tc.tile_pool(name="outp", bufs=4))
            nump = pb.enter_context(tc.tile_pool(name="nump", bufs=4, space="PSUM"))
            vsps = pb.enter_context(tc.tile_pool(name="vsps", bufs=2, space="PSUM"))

            def sweep(g, nums):
                for rc in range(n_cores):
                    vp3 = vpg[g][rc].rearrange("p (t d) -> p t d", t=TL)
                    for pr in range(TL // 2):
                        rgb = (rc * TL + 2 * pr) * B
                        stp3 = (st[:, rgb: rgb + 2 * B]
                                .rearrange("p (two b) -> p two b", two=2))
                        rhs = vp3[:, 2 * pr:2 * pr + 2, :]
                        for ic in range(TL):
                            nc.tensor.matmul(
                                nums[ic][:],
                                stp3[:, :, ic * P:(ic + 1) * P],
                                rhs,
                                start=(rc == 0 and pr == 0), stop=False,
                                perf_mode=mybir.MatmulPerfMode.DoubleRow,
                            )

            def close_group(g, nums):
                for ic in range(TL):
                    nc.tensor.matmul(
                        nums[ic][:], suf_sb[:, ic * P:(ic + 1) * P],
                        vs_sb[:, g * D2:(g + 1) * D2],
                        start=False, stop=True,
                    )
                    ot = outp.tile([P, D2], F32, tag="ot", name=f"ot{g}_{ic}")
                    if ic % 2 == 0:
                        nc.vector.tensor_scalar(ot[:], nums[ic][:],
                                                recip[:, ic:ic + 1], None, op0=ALU.mult)
                    else:
                        nc.scalar.activation(ot[:], nums[ic][:], AF.Copy,
                                             scale=recip[:, ic:ic + 1])
                    (nc.sync if ic % 2 == 0 else nc.scalar).dma_start(
                        out.ap()[ic * P:(ic + 1) * P, g * D2:(g + 1) * D2], ot[:])

            nums0 = [nump.tile([P, D2], F32, tag="num", name=f"num_ps0_{ic}")
                     for ic in range(TL)]
            sweep(0, nums0)

            # VS = xrs.T-mm(wv)  [T, D] (gathered x row sums, re-strided)
            nc.vector.tensor_copy(
                xrs_g.rearrange("p (dc c t) -> p dc c t", dc=DC, c=n_cores),
                xrs_s.rearrange("p (c dc t) -> p dc c t", c=n_cores, dc=DC))
            for nh in range(NH):
                vs_ps = vsps.tile([T, W], F32, tag="vs")
                for dci in range(DC):
                    nc.tensor.matmul(
                        vs_ps[:],
                        xrs_g[:, dci * T:(dci + 1) * T],
                        wv_sb[:, dci * D + nh * W: dci * D + (nh + 1) * W],
                        start=(dci == 0), stop=(dci == DC - 1),
                    )
                nc.vector.tensor_copy(vs_sb[:, nh * W:(nh + 1) * W], vs_ps[:])

            close_group(0, nums0)
            nums1 = [nump.tile([P, D2], F32, tag="num", name=f"num_ps1_{ic}")
                     for ic in range(TL)]
            sweep(1, nums1)
            close_group(1, nums1)

    nc.compile()
    return nc


def make_in_maps(x_full, wq, wk, wv, n_cores=8):
    import ml_dtypes
    bf = lambda a: np.ascontiguousarray(a).astype(ml_dtypes.bfloat16)
    f8 = lambda a: np.ascontiguousarray(a).astype(ml_dtypes.float8_e4m3)
    SEQ, D = x_full.shape
    T = SEQ // P
    B = SEQ // n_cores
    consts = make_consts(T)
    wq8 = f8(wq * 8.0)
    wkt2 = f8(wk.T * 2.0)
    wvb = bf(wv)
    in_maps = []
    for c in range(n_cores):
        m = {"x": bf(x_full[c * B:(c + 1) * B]),
             "wq": wq8, "wk": wkt2, "wv": wvb}
        m.update(consts)
        in_maps.append(m)
    return in_maps


def algo_ref(x, wq, wk, wv):
    """Numpy float64 reference of the restructured math (for validation)."""
    x = x.astype(np.float64)
    q2 = (x @ wq.astype(np.float64)) @ wk.astype(np.float64).T
    s = np.exp(q2 @ x.T / np.sqrt(x.shape[1]))
    Vc = np.cumsum(x @ wv.astype(np.float64), axis=0)
    num = s @ Vc
    den = s @ (np.arange(x.shape[0]) + 1.0)
    return (num / den[:, None]).astype(np.float32)


# ----------------------------------------------------------------------------
# Harness entry point: full (unsharded) inputs -> full output.
# ----------------------------------------------------------------------------
SEQ, D_IN, N_CORES = 4096, 1024, 8
_built = {}


def _get_nc(SEQ_=SEQ, D_=D_IN, n_cores=N_CORES):
    key = (SEQ_, D_, n_cores)
    if key not in _built:
        _built[key] = build(SEQ=SEQ_, D=D_, n_cores=n_cores)
    return _built[key]


def run(x, wq, wk, wv, trace=False, **spmd_kwargs):
    from concourse.bass_utils import run_bass_kernel_spmd

    x = np.ascontiguousarray(np.asarray(x, dtype=np.float32))
    wq = np.ascontiguousarray(np.asarray(wq, dtype=np.float32))
    wk = np.ascontiguousarray(np.asarray(wk, dtype=np.float32))
    wv = np.ascontiguousarray(np.asarray(wv, dtype=np.float32))
    n_cores = N_CORES
    nc = _get_nc(x.shape[0], x.shape[1], n_cores)
    in_maps = make_in_maps(x, wq, wk, wv, n_cores=n_cores)
    res = run_bass_kernel_spmd(nc, in_maps, list(range(n_cores)),
                               trace=trace, **spmd_kwargs)
    out = np.concatenate([res.results[c]["out"] for c in range(n_cores)], axis=0)
    return out, res


def kernel(x, wq, wk, wv):
    out, _ = run(x, wq, wk, wv, trace=False)
    return out


# revision 47
# speedup vs baseline: 1.3022x; 1.0159x over previous
"""Bass/Tile TRN2 kernel for nn_CausalAttention (softmax + tril-matmul renorm).

Math restructuring (per core, row block of B = SEQ/n_cores rows):
    q = x @ wq ; k = x @ wk ; v = x @ wv
    z = q @ k.T / sqrt(D) ;  s = exp(z)              (softmax norm cancels below)
    masked[i,j] = sum_{t>=j} s[i,t]                  (suffix sum == s @ tril)
    out = (masked @ v) / rowsum(masked)

v2 identities (vs the tril/suffix formulation):
    masked @ v       == s @ cumsum(v)                 -> contract s with prefix-V
    rowsum(masked)   == s @ (t+1)                     -> one weight column
    z = q @ k.T      == ((x@wq) @ wk.T) @ x.T         -> gather RAW x, not K

so the AllGather input (x.T in fp8) is ready ~6us into the kernel instead of
after a full projection, and the O(S^2) tril matmuls + psum copies vanish.

Per-tile decomposition (tile r of 128 keys, T tiles):
    Vc[rP+j] = Vc0_r[j] + sum_{r'<r} VS[r']           (within-tile prefix + offsets)
    numT = sum_r Vc0_r.T-mm(st_r) + VS.T-mm(SUF)      SUF[r] = sum_{r'>r} CS[r']
    den[i] = sum_t (t+1) s[t,i]                       (selector-pair matmul w/ CS)

Layouts: everything transposed ([feature/key on partitions, query on free]).
Prefix-x trick: the x-transpose matmuls use rhs=[I | U] (U=upper-tri ones) so a
single pass yields both x.T (fp8, scores+gather) and prefix-x.T (bf16, V path);
xrs (tile row-sums of x) is the last U-column, read from PSUM in f32.

Collectives: cc1 = AllGather(x.T fp8) triggered right after the transposes;
cc2 = AllGather(Vc0 fp8 + xrs bf16) after the V projection. Both on the
otherwise-empty GpSimd queue so nothing delays the trigger.
"""
import numpy as np
from contextlib import ExitStack

import concourse.bass as bass
import concourse.tile as tile
from concourse import bacc, mybir

F32 = mybir.dt.float32
BF16 = mybir.dt.bfloat16
FP8 = mybir.dt.float8e4
U8 = mybir.dt.uint8
AX = mybir.AxisListType
AF = mybir.ActivationFunctionType
ALU = mybir.AluOpType

P = 128
MB = 48          # selector pair block half-width (padded for DoubleRow step%16)


def make_consts(T):
    iu = np.concatenate([np.eye(P, dtype=np.float32),
                         np.triu(np.ones((P, P), np.float32))], axis=1)
    stril = np.tri(T, T, -1, dtype=np.float32)  # [r', r] = 1 if r' > r
    # selector pair blocks (DoubleRow over tile pairs a=2pr, b=2pr+1):
    # ko0 col a / ko1 col b = 1 (per-tile colsum -> CS rows); col 32 = den
    # weight (t+1)/32, pinned to partition 32 for the later row extraction.
    NPAIR = T // 2
    selp = np.zeros((P, NPAIR * 2 * MB), np.float32)
    for pr in range(NPAIR):
        a, b = 2 * pr, 2 * pr + 1
        blk = pr * 2 * MB
        selp[:, blk + a] = 1.0
        selp[:, blk + 32] = (P * a + np.arange(P) + 1.0) / 32.0
        selp[:, blk + MB + b] = 1.0
        selp[:, blk + MB + 32] = (P * b + np.arange(P) + 1.0) / 32.0
    ident = np.eye(P, dtype=np.float32)
    import ml_dtypes
    bf = lambda a: a.astype(ml_dtypes.bfloat16)
    f8 = lambda a: a.astype(ml_dtypes.float8_e4m3)
    return dict(c_iu=bf(iu), c_stril=bf(stril), c_selp=f8(selp), c_ident=ident)


def build(SEQ=4096, D=1024, n_cores=8):
    T = SEQ // P           # global 128-key tiles
    TL = T // n_cores      # local tiles per core
    B = P * TL             # rows per core
    DC = D // P            # feature chunks
    W = min(512, D)        # moving free width for D-wide matmuls
    NH = D // W
    NPAIR = T // 2
    assert B <= 512 and T <= P and D % W == 0 and SEQ % (P * n_cores) == 0
    # wq prescaled x8, wk.T prescaled x2 -> z = 512 * z_true
    scale = float(1.0 / np.sqrt(D) / 16.0)
    EXPB = float(-np.log(16.0))   # st = s/16 keeps fp8e4 range safe

    nc = bacc.Bacc("TRN2", target_bir_lowering=False, debug=False, num_devices=n_cores)

    x = nc.dram_tensor("x", [B, D], BF16, kind="ExternalInput")
    wq_d = nc.dram_tensor("wq", [D, D], FP8, kind="ExternalInput")
    wkt_d = nc.dram_tensor("wk", [D, D], FP8, kind="ExternalInput")   # wk.T * 2
    wv_d = nc.dram_tensor("wv", [D, D], BF16, kind="ExternalInput")
    c_iu = nc.dram_tensor("c_iu", [P, 2 * P], BF16, kind="ExternalInput")
    c_stril = nc.dram_tensor("c_stril", [T, T], BF16, kind="ExternalInput")
    c_selp = nc.dram_tensor("c_selp", [P, NPAIR * 2 * MB], FP8, kind="ExternalInput")
    c_ident = nc.dram_tensor("c_ident", [P, P], F32, kind="ExternalInput")
    out = nc.dram_tensor("out", [B, D], F32, kind="ExternalOutput")

    # all four collectives are half-splits so the meshes pipeline with compute:
    # cc1a/cc1b = x.T key-halves (2 tiles each); cc2a/cc2b = Vc0 d-halves
    # (cc2a also carries xrs).
    TLH = TL // 2          # local tiles per key-half
    B2 = P * TLH
    KH = D * B2            # cc1{a,b}: xT8 half [P, DC*B2] fp8, flat (p k)
    D2 = D // 2
    VNB = B * D2           # cc2{a,b}: Vc0 d-half [P, TL*D2] fp8e4
    XRB = 2 * D * TL       # cc2a extra: xrs region [P, DC*TL] bf16

    with tile.TileContext(nc) as tc, ExitStack() as top:
        dram = top.enter_context(tc.tile_pool(name="dram", bufs=1, space="DRAM"))
        cc1a_in = dram.tile([KH], FP8)
        cc1a_out = dram.tile([n_cores, KH], FP8, addr_space="Shared")
        cc1b_in = dram.tile([KH], FP8)
        cc1b_out = dram.tile([n_cores, KH], FP8, addr_space="Shared")
        cc2a_in = dram.tile([VNB + XRB], U8)
        cc2a_out = dram.tile([n_cores, VNB + XRB], U8, addr_space="Shared")
        cc2b_in = dram.tile([VNB], U8)
        cc2b_out = dram.tile([n_cores, VNB], U8, addr_space="Shared")

        consts = top.enter_context(tc.tile_pool(name="consts", bufs=1))
        iu_sb = consts.tile([P, 2 * P], BF16)
        nc.scalar.dma_start(iu_sb[:], c_iu.ap())
        stril_sb = consts.tile([T, T], BF16)
        nc.scalar.dma_start(stril_sb[:], c_stril.ap())
        selp_sb = consts.tile([P, NPAIR * 2 * MB], FP8)
        nc.scalar.dma_start(selp_sb[:], c_selp.ap())
        ident_sb = consts.tile([P, P], F32)
        nc.scalar.dma_start(ident_sb[:], c_ident.ap())
        expb_sb = consts.tile([P, 1], F32)
        nc.vector.memset(expb_sb[:], EXPB)

        persist = top.enter_context(tc.tile_pool(name="persist", bufs=1))
        q2T = persist.tile([P, DC * B], FP8)         # (q @ wk.T).T row block
        st = persist.tile([P, T * B], FP8)           # exp(scores)/16, transposed
        vs_sb = persist.tile([T, D], BF16)           # per-tile V colsums
        xrs_s = persist.tile([P, n_cores * DC * TL], BF16)  # gathered x row sums [p,(c dc t)]
        xrs_g = persist.tile([P, DC * T], BF16)             # re-strided to [p,(dc r)]
        suf_sb = persist.tile([T, B], BF16)
        cs_sb = persist.tile([T, B], BF16)
        recip = persist.tile([P, TL], F32)
        dennat = persist.tile([P, TL], F32)
        den_pad = persist.tile([P, B], F32)
        wv_sb = persist.tile([P, DC * D], BF16)
        vpg = [[persist.tile([P, TL * D2], FP8, name=f"vp{g}_{rc}")
                for rc in range(n_cores)] for g in range(2)]

        # ------------- stage 1: transposes, gather-x, projections -------------
        with ExitStack() as s1:
            xpool = s1.enter_context(tc.tile_pool(name="xload", bufs=6))
            xTp = s1.enter_context(tc.tile_pool(name="xT", bufs=1))
            xT8 = xTp.tile([P, DC * B], FP8)     # x.T   (scores lhsT + cc1 input)
            xcT = xTp.tile([P, DC * B], BF16)    # prefix-x.T (V path)
            xrs_f = xTp.tile([P, DC * TL], F32)
            xrs_bf = xTp.tile([P, DC * TL], BF16)

            wpool = s1.enter_context(tc.tile_pool(name="w", bufs=1))
            wq_sb = wpool.tile([P, DC * D], FP8)
            wkt_sb = wpool.tile([P, DC * D], FP8)
            qT = wpool.tile([P, DC * B], FP8)

            trps = s1.enter_context(tc.tile_pool(name="trps", bufs=2, space="PSUM"))
            # x.T first (I pass) -> cc1 trigger ASAP; prefix-x.T (U pass) after
            xts = []
            for tcc in range(TL):
                xt_ = xpool.tile([P, D], BF16, tag="x", name=f"xt_{tcc}")
                (nc.sync if tcc < 2 else nc.scalar).dma_start(
                    xt_[:], x.ap()[tcc * P:(tcc + 1) * P, :])
                xts.append(xt_)
            cc1av = cc1a_in[0:KH].rearrange("(p k) -> p k", p=P)
            cc1bv = cc1b_in[0:KH].rearrange("(p k) -> p k", p=P)
            for dc in range(DC):
                psI = trps.tile([P, B], F32, tag="trI")
                for tcc in range(TL):
                    nc.tensor.matmul(psI[:, tcc * P:(tcc + 1) * P],
                                     xts[tcc][:, dc * P:(dc + 1) * P], iu_sb[:, 0:P],
                                     start=True, stop=True)
                (nc.vector.tensor_copy if dc % 2 == 0 else nc.scalar.copy)(
                    xT8[:, dc * B:(dc + 1) * B], psI[:])
                # stream x.T key-halves to the collective inputs as they land
                nc.sync.dma_start(cc1av[:, dc * B2:(dc + 1) * B2],
                                  xT8[:, dc * B:dc * B + B2])
                nc.sync.dma_start(cc1bv[:, dc * B2:(dc + 1) * B2],
                                  xT8[:, dc * B + B2:(dc + 1) * B])

            # gather x.T immediately (GpSimd queue is otherwise empty)
            nc.gpsimd.collective_compute(
                "AllGather", ALU.bypass,
                replica_groups=[list(range(n_cores))],
                ins=[cc1a_in.opt()], outs=[cc1a_out.opt()],
            )
            nc.gpsimd.collective_compute(
                "AllGather", ALU.bypass,
                replica_groups=[list(range(n_cores))],
                ins=[cc1b_in.opt()], outs=[cc1b_out.opt()],
            )

            for dc in range(DC):
                psU = trps.tile([P, B], F32, tag="trU")
                for tcc in range(TL):
                    nc.tensor.matmul(psU[:, tcc * P:(tcc + 1) * P],
                                     xts[tcc][:, dc * P:(dc + 1) * P], iu_sb[:, P:2 * P],
                                     start=True, stop=True)
                (nc.scalar.copy if dc % 2 == 0 else nc.vector.tensor_copy)(
                    xcT[:, dc * B:(dc + 1) * B], psU[:])
                nc.vector.tensor_copy(
                    xrs_f[:, dc * TL:(dc + 1) * TL]
                    .rearrange("p (t one) -> p t one", one=1),
                    psU.rearrange("p (t j) -> p t j", j=P)[:, :, P - 1:P])

            # weights (HWDGE queues; nothing gates the collective).
            # wv first: the V path gates cc2, which gates phase B.
            for dc in range(DC):
                nc.scalar.dma_start(wv_sb[:, dc * D:(dc + 1) * D], wv_d.ap()[dc * P:(dc + 1) * P, :])
            for dc in range(DC):
                nc.sync.dma_start(wq_sb[:, dc * D:(dc + 1) * D], wq_d.ap()[dc * P:(dc + 1) * P, :])
            for dc in range(DC):
                nc.scalar.dma_start(wkt_sb[:, dc * D:(dc + 1) * D], wkt_d.ap()[dc * P:(dc + 1) * P, :])

            pps = s1.enter_context(tc.tile_pool(name="pps", bufs=4, space="PSUM"))

            # Vc0 row blocks (within-tile prefix V, from prefix-x.T) -> cc2
            cc2av = cc2a_in[0:VNB].rearrange("(p k) -> p k", p=P)
            cc2bv = cc2b_in[0:VNB].rearrange("(p k) -> p k", p=P)
            for nh in range(NH):
                for tcc in range(TL):
                    vl = xpool.tile([P, D2], FP8, tag="vl")
                    v_ps = pps.tile([P, W], F32, tag="pp", name="v_ps")
                    for dci in range(DC):
                        nc.tensor.matmul(
                            v_ps[:],
                            xcT[:, dci * B + tcc * P: dci * B + (tcc + 1) * P],
                            wv_sb[:, dci * D + nh * W: dci * D + (nh + 1) * W],
                            start=(dci == 0), stop=(dci == DC - 1),
                        )
                    (nc.vector.tensor_copy if tcc % 2 == 0 else nc.scalar.copy)(
                        vl[:], v_ps[:])
                    nc.gpsimd.dma_start(
                        (cc2av if nh == 0 else cc2bv)[:, tcc * D2:(tcc + 1) * D2],
                        vl[:].bitcast(U8))
                if nh == 0:
                    # per-tile x row sums (f32 exact, from the U-part last col)
                    nc.vector.tensor_copy(xrs_bf[:], xrs_f[:])
                    cc2x = cc2a_in[VNB:VNB + XRB].rearrange("(p k) -> p k", p=P)
                    nc.gpsimd.dma_start(cc2x[:, :], xrs_bf[:].bitcast(U8))

            # qT = (x @ wq).T  then  q2T = (q @ wk.T).T   (both fp8 DoubleRow)
            wq3 = wq_sb.rearrange("p (dc d) -> p dc d", dc=DC)
            wkt3 = wkt_sb.rearrange("p (dc d) -> p dc d", dc=DC)
            xT83 = xT8.rearrange("p (dc b) -> p dc b", dc=DC)
            for dco in range(DC):
                q_ps = pps.tile([P, B], F32, tag="pp", name="q_ps")
                for pp_ in range(DC // 2):
                    nc.tensor.matmul(
                        q_ps[:],
                        wq3[:, 2 * pp_:2 * pp_ + 2, dco * P:(dco + 1) * P],
                        xT83[:, 2 * pp_:2 * pp_ + 2, :],
                        start=(pp_ == 0), stop=(pp_ == DC // 2 - 1),
                        perf_mode=mybir.MatmulPerfMode.DoubleRow,
                    )
                nc.vector.tensor_copy(qT[:, dco * B:(dco + 1) * B], q_ps[:])
            qT3 = qT.rearrange("p (dc b) -> p dc b", dc=DC)
            for dco in range(DC):
                q2_ps = pps.tile([P, B], F32, tag="pp", name="q2_ps")
                for pp_ in range(DC // 2):
                    nc.tensor.matmul(
                        q2_ps[:],
                        wkt3[:, 2 * pp_:2 * pp_ + 2, dco * P:(dco + 1) * P],
                        qT3[:, 2 * pp_:2 * pp_ + 2, :],
                        start=(pp_ == 0), stop=(pp_ == DC // 2 - 1),
                        perf_mode=mybir.MatmulPerfMode.DoubleRow,
                    )
                nc.vector.tensor_copy(q2T[:, dco * B:(dco + 1) * B], q2_ps[:])

        # second collective pair: gather Vc0 halves (+xrs with the first)
        nc.gpsimd.collective_compute(
            "AllGather", ALU.bypass,
            replica_groups=[list(range(n_cores))],
            ins=[cc2a_in.opt()], outs=[cc2a_out.opt()],
        )
        nc.gpsimd.collective_compute(
            "AllGather", ALU.bypass,
            replica_groups=[list(range(n_cores))],
            ins=[cc2b_in.opt()], outs=[cc2b_out.opt()],
        )
        # gathered x row sums: one DMA, 3D AP over source cores
        nc.sync.dma_start(
            xrs_s[:].bitcast(U8).rearrange("p (c k) -> p c k", c=n_cores),
            cc2a_out[0:n_cores, VNB:VNB + XRB].rearrange("c (p k) -> p c k", p=P))
        # prefetch gathered Vc0 blocks (fires as soon as each gather lands)
        for rc in range(n_cores):
            nc.gpsimd.dma_start(
                vpg[0][rc][:].bitcast(U8),
                cc2a_out[rc, 0:VNB].rearrange("(p k) -> p k", p=P))

        # ------------------- phase A: scores / exp / CS+den -------------------
        with ExitStack() as pa:
            ktp = pa.enter_context(tc.tile_pool(name="kt", bufs=3))
            ztp = pa.enter_context(tc.tile_pool(name="zt", bufs=3, space="PSUM"))
            csp = pa.enter_context(tc.tile_pool(name="csp", bufs=1, space="PSUM"))
            sfp = pa.enter_context(tc.tile_pool(name="sfp", bufs=2, space="PSUM"))
            cs_ps = csp.tile([33, B], F32)

            q2T3 = q2T.rearrange("p (dc b) -> p dc b", dc=DC)
            pi = 0
            for h in range(2):
                cch = cc1a_out if h == 0 else cc1b_out
                for rc in range(n_cores):
                    ktc = ktp.tile([P, DC * B2], FP8, tag="kt")
                    nc.sync.dma_start(ktc[:], cch[rc, 0:KH].rearrange("(p k) -> p k", p=P))
                    ktc3 = ktc.rearrange("p (dc i) -> p dc i", dc=DC)
                    for sub2 in range(TLH):
                        rg = rc * TL + h * TLH + sub2
                        zt = ztp.tile([P, B], F32, tag="zt")
                        for pp in range(DC // 2):
                            nc.tensor.matmul(
                                zt[:],
                                ktc3[:, 2 * pp:2 * pp + 2, sub2 * P:(sub2 + 1) * P],
                                q2T3[:, 2 * pp:2 * pp + 2, :],
                                start=(pp == 0), stop=(pp == DC // 2 - 1),
                                perf_mode=mybir.MatmulPerfMode.DoubleRow,
                            )
                        nc.scalar.activation(st[:, rg * B:(rg + 1) * B], zt[:],
                                             AF.Exp, bias=expb_sb[:], scale=scale)
                        if sub2 % 2 == 1:
                            pr = rg // 2
                            lp = (selp_sb[:, pr * 2 * MB:(pr + 1) * 2 * MB]
                                  .rearrange("p (two m) -> p two m", two=2)[:, :, 0:33])
                            rp = (st[:, (rg - 1) * B:(rg + 1) * B]
                                  .rearrange("p (two b) -> p two b", two=2))
                            nc.tensor.matmul(
                                cs_ps[:], lp, rp,
                                start=(pi == 0), stop=(pi == NPAIR - 1),
                                perf_mode=mybir.MatmulPerfMode.DoubleRow,
                            )
                            pi += 1

            for rc in range(n_cores):
                nc.scalar.dma_start(
                    vpg[1][rc][:].bitcast(U8),
                    cc2b_out[rc, 0:VNB].rearrange("(p k) -> p k", p=P))

            nc.vector.tensor_copy(cs_sb[:], cs_ps[0:T, :])
            nc.vector.memset(den_pad[:], 0.0)
            nc.vector.tensor_copy(den_pad[32:33, :], cs_ps[32:33, :])
            suf_ps = sfp.tile([T, B], F32)
            nc.tensor.matmul(suf_ps[:], stril_sb[:], cs_sb[:], start=True, stop=True)
            nc.scalar.copy(suf_sb[:], suf_ps[:])

            # 0.03125/den now, so the phase-B epilogues are never gated on it
            for sub in range(TL):
                dps = sfp.tile([P, P], F32, tag="dtp")
                nc.tensor.transpose(dps[:], den_pad[:, sub * P:(sub + 1) * P], ident_sb[:])
                nc.vector.tensor_scalar(dennat[:, sub:sub + 1], dps[:, 32:33], 32.0,
                                        None, op0=ALU.mult)
            nc.vector.reciprocal(recip[:], dennat[:])

        # ------------- phase B: num accumulation (natural layout) -------------
        # num[i, d] = sum_t st[t, i] Vc0[t, d] + sum_r SUF[r, i] VS[r, d];
        # st tile pairs are the stationary operand, Vc0 pairs the moving one,
        # so the output lands directly in [query, feature] layout: no
        # transposes, and the den scale is a per-partition scalar multiply.
        # VS is computed between the g0 sweep and its closes so the PE never
        # stalls on the xrs gather at the phase boundary.
        with ExitStack() as pb:
            outp = pb.enter_context(tc.tile_pool(name="outp", bufs=4))
            nump = pb.enter_context(tc.tile_pool(name="nump", bufs=4, space="PSUM"))
            vsps = pb.enter_context(tc.tile_pool(name="vsps", bufs=2, space="PSUM"))

            def sweep(g, nums):
                for rc in range(n_cores):
                    vp3 = vpg[g][rc].rearrange("p (t d) -> p t d", t=TL)
                    for pr in range(TL // 2):
                        rgb = (rc * TL + 2 * pr) * B
                        stp3 = (st[:, rgb: rgb + 2 * B]
                                .rearrange("p (two b) -> p two b", two=2))
                        rhs = vp3[:, 2 * pr:2 * pr + 2, :]
                        for ic in range(TL):
                            nc.tensor.matmul(
                                nums[ic][:],
                                stp3[:, :, ic * P:(ic + 1) * P],
                                rhs,
                                start=(rc == 0 and pr == 0), stop=False,
                                perf_mode=mybir.MatmulPerfMode.DoubleRow,
                            )

            def close_group(g, nums):
                for ic in range(TL):
                    nc.tensor.matmul(
                        nums[ic][:], suf_sb[:, ic * P:(ic + 1) * P],
                        vs_sb[:, g * D2:(g + 1) * D2],
                        start=False, stop=True,
                    )
                    ot = outp.tile([P, D2], F32, tag="ot", name=f"ot{g}_{ic}")
                    if ic % 2 == 0:
                        nc.vector.tensor_scalar(ot[:], nums[ic][:],
                                                recip[:, ic:ic + 1], None, op0=ALU.mult)
                    else:
                        nc.scalar.activation(ot[:], nums[ic][:], AF.Copy,
                                             scale=recip[:, ic:ic + 1])
                    (nc.sync if ic % 2 == 0 else nc.scalar).dma_start(
                        out.ap()[ic * P:(ic + 1) * P, g * D2:(g + 1) * D2], ot[:])

            nums0 = [nump.tile([P, D2], F32, tag="num", name=f"num_ps0_{ic}")
                     for ic in range(TL)]
            sweep(0, nums0)

            # VS = xrs.T-mm(wv)  [T, D] (gathered x row sums, re-strided)
            nc.vector.tensor_copy(
                xrs_g.rearrange("p (dc c t) -> p dc c t", dc=DC, c=n_cores),
                xrs_s.rearrange("p (c dc t) -> p dc c t", c=n_cores, dc=DC))
            for nh in range(NH):
                vs_ps = vsps.tile([T, W], F32, tag="vs")
                for dci in range(DC):
                    nc.tensor.matmul(
                        vs_ps[:],
                        xrs_g[:, dci * T:(dci + 1) * T],
                        wv_sb[:, dci * D + nh * W: dci * D + (nh + 1) * W],
                        start=(dci == 0), stop=(dci == DC - 1),
                    )
                nc.vector.tensor_copy(vs_sb[:, nh * W:(nh + 1) * W], vs_ps[:])

            close_group(0, nums0)
            nums1 = [nump.tile([P, D2], F32, tag="num", name=f"num_ps1_{ic}")
                     for ic in range(TL)]
            sweep(1, nums1)
            close_group(1, nums1)

    nc.compile()
    return nc


def make_in_maps(x_full, wq, wk, wv, n_cores=8):
    import ml_dtypes
    bf = lambda a: np.ascontiguousarray(a).astype(ml_dtypes.bfloat16)
    f8 = lambda a: np.ascontiguousarray(a).astype(ml_dtypes.float8_e4m3)
    SEQ, D = x_full.shape
    T = SEQ // P
    B = SEQ // n_cores
    consts = make_consts(T)
    wq8 = f8(wq * 8.0)
    wkt2 = f8(wk.T * 2.0)
    wvb = bf(wv)
    in_maps = []
    for c in range(n_cores):
        m = {"x": bf(x_full[c * B:(c + 1) * B]),
             "wq": wq8, "wk": wkt2, "wv": wvb}
        m.update(consts)
        in_maps.append(m)
    return in_maps


def algo_ref(x, wq, wk, wv):
    """Numpy float64 reference of the restructured math (for validation)."""
    x = x.astype(np.float64)
    q2 = (x @ wq.astype(np.float64)) @ wk.astype(np.float64).T
    s = np.exp(q2 @ x.T / np.sqrt(x.shape[1]))
    Vc = np.cumsum(x @ wv.astype(np.float64), axis=0)
    num = s @ Vc
    den = s @ (np.arange(x.shape[0]) + 1.0)
    return (num / den[:, None]).astype(np.float32)


# ----------------------------------------------------------------------------
# Harness entry point: full (unsharded) inputs -> full output.
# ----------------------------------------------------------------------------
SEQ, D_IN, N_CORES = 4096, 1024, 8
_built = {}


def _get_nc(SEQ_=SEQ, D_=D_IN, n_cores=N_CORES):
    key = (SEQ_, D_, n_cores)
    if key not in _built:
        _built[key] = build(SEQ=SEQ_, D=D_, n_cores=n_cores)
    return _built[key]


def run(x, wq, wk, wv, trace=False, **spmd_kwargs):
    from concourse.bass_utils import run_bass_kernel_spmd

    x = np.ascontiguousarray(np.asarray(x, dtype=np.float32))
    wq = np.ascontiguousarray(np.asarray(wq, dtype=np.float32))
    wk = np.ascontiguousarray(np.asarray(wk, dtype=np.float32))
    wv = np.ascontiguousarray(np.asarray(wv, dtype=np.float32))
    n_cores = N_CORES
    nc = _get_nc(x.shape[0], x.shape[1], n_cores)
    in_maps = make_in_maps(x, wq, wk, wv, n_cores=n_cores)
    res = run_bass_kernel_spmd(nc, in_maps, list(range(n_cores)),
                               trace=trace, **spmd_kwargs)
    out = np.concatenate([res.results[c]["out"] for c in range(n_cores)], axis=0)
    return out, res


def kernel(x, wq, wk, wv):
    out, _ = run(x, wq, wk, wv, trace=False)
    return out


# revision 54
# speedup vs baseline: 1.3544x; 1.0401x over previous
"""Bass/Tile TRN2 kernel for nn_CausalAttention (softmax + tril-matmul renorm).

Math restructuring (per core, row block of B = SEQ/n_cores rows):
    q = x @ wq ; k = x @ wk ; v = x @ wv
    z = q @ k.T / sqrt(D) ;  s = exp(z)              (softmax norm cancels below)
    masked[i,j] = sum_{t>=j} s[i,t]                  (suffix sum == s @ tril)
    out = (masked @ v) / rowsum(masked)

v2 identities (vs the tril/suffix formulation):
    masked @ v       == s @ cumsum(v)                 -> contract s with prefix-V
    rowsum(masked)   == s @ (t+1)                     -> one weight column
    z = q @ k.T      == ((x@wq) @ wk.T) @ x.T         -> gather RAW x, not K

so the AllGather input (x.T in fp8) is ready ~6us into the kernel instead of
after a full projection, and the O(S^2) tril matmuls + psum copies vanish.

Per-tile decomposition (tile r of 128 keys, T tiles):
    Vc[rP+j] = Vc0_r[j] + sum_{r'<r} VS[r']           (within-tile prefix + offsets)
    numT = sum_r Vc0_r.T-mm(st_r) + VS.T-mm(SUF)      SUF[r] = sum_{r'>r} CS[r']
    den[i] = sum_t (t+1) s[t,i]                       (selector-pair matmul w/ CS)

Layouts: everything transposed ([feature/key on partitions, query on free]).
Prefix-x trick: the x-transpose matmuls use rhs=[I | U] (U=upper-tri ones) so a
single pass yields both x.T (fp8, scores+gather) and prefix-x.T (bf16, V path);
xrs (tile row-sums of x) is the last U-column, read from PSUM in f32.

Collectives: cc1 = AllGather(x.T fp8) triggered right after the transposes;
cc2 = AllGather(Vc0 fp8 + xrs bf16) after the V projection. Both on the
otherwise-empty GpSimd queue so nothing delays the trigger.
"""
import numpy as np
from contextlib import ExitStack

import concourse.bass as bass
import concourse.tile as tile
from concourse import bacc, mybir

F32 = mybir.dt.float32
BF16 = mybir.dt.bfloat16
FP8 = mybir.dt.float8e4
U8 = mybir.dt.uint8
AX = mybir.AxisListType
AF = mybir.ActivationFunctionType
ALU = mybir.AluOpType

P = 128
MB = 48          # selector pair block half-width (padded for DoubleRow step%16)


def make_consts(T):
    iu = np.concatenate([np.eye(P, dtype=np.float32),
                         np.triu(np.ones((P, P), np.float32))], axis=1)
    stril = np.tri(T, T, -1, dtype=np.float32)  # [r', r] = 1 if r' > r
    # selector pair blocks (DoubleRow over tile pairs a=2pr, b=2pr+1):
    # ko0 col a / ko1 col b = 1 (per-tile colsum -> CS rows); col 32 = den
    # weight (t+1)/32, pinned to partition 32 for the later row extraction.
    NPAIR = T // 2
    selp = np.zeros((P, NPAIR * 2 * MB), np.float32)
    for pr in range(NPAIR):
        a, b = 2 * pr, 2 * pr + 1
        blk = pr * 2 * MB
        selp[:, blk + a] = 1.0
        selp[:, blk + 32] = (P * a + np.arange(P) + 1.0) / 32.0
        selp[:, blk + MB + b] = 1.0
        selp[:, blk + MB + 32] = (P * b + np.arange(P) + 1.0) / 32.0
    ident = np.eye(P, dtype=np.float32)
    import ml_dtypes
    bf = lambda a: a.astype(ml_dtypes.bfloat16)
    f8 = lambda a: a.astype(ml_dtypes.float8_e4m3)
    return dict(c_iu=bf(iu), c_stril=bf(stril), c_selp=f8(selp), c_ident=ident)


def build(SEQ=4096, D=1024, n_cores=8):
    T = SEQ // P           # global 128-key tiles
    TL = T // n_cores      # local tiles per core
    B = P * TL             # rows per core
    DC = D // P            # feature chunks
    W = min(512, D)        # moving free width for D-wide matmuls
    NH = D // W
    NPAIR = T // 2
    assert B <= 512 and T <= P and D % W == 0 and SEQ % (P * n_cores) == 0
    # wq prescaled x8, wk.T prescaled x2 -> z = 512 * z_true
    scale = float(1.0 / np.sqrt(D) / 16.0)
    EXPB = float(-np.log(16.0))   # st = s/16 keeps fp8e4 range safe

    nc = bacc.Bacc("TRN2", target_bir_lowering=False, debug=False, num_devices=n_cores)

    x = nc.dram_tensor("x", [B, D], BF16, kind="ExternalInput")
    wq_d = nc.dram_tensor("wq", [D, D], FP8, kind="ExternalInput")
    wkt_d = nc.dram_tensor("wk", [D, D], FP8, kind="ExternalInput")   # wk.T * 2
    wv_d = nc.dram_tensor("wv", [D, D], BF16, kind="ExternalInput")
    c_iu = nc.dram_tensor("c_iu", [P, 2 * P], BF16, kind="ExternalInput")
    c_stril = nc.dram_tensor("c_stril", [T, T], BF16, kind="ExternalInput")
    c_selp = nc.dram_tensor("c_selp", [P, NPAIR * 2 * MB], FP8, kind="ExternalInput")
    c_ident = nc.dram_tensor("c_ident", [P, P], F32, kind="ExternalInput")
    out = nc.dram_tensor("out", [B, D], F32, kind="ExternalOutput")

    # all four collectives are half-splits so the meshes pipeline with compute:
    # cc1a/cc1b = x.T key-halves (2 tiles each); cc2a/cc2b = Vc0 d-halves
    # (cc2a also carries xrs).
    TLH = TL // 2          # local tiles per key-half
    B2 = P * TLH
    KH = D * B2            # cc1{a,b}: xT8 half [P, DC*B2] fp8, flat (p k)
    D2 = D // 2
    VNB = B * D2           # cc2{a,b}: Vc0 d-half [P, TL*D2] fp8e4
    XRB = 2 * D * TL       # cc2a extra: xrs region [P, DC*TL] bf16

    with tile.TileContext(nc) as tc, ExitStack() as top:
        dram = top.enter_context(tc.tile_pool(name="dram", bufs=1, space="DRAM"))
        cc1a_in = dram.tile([KH], FP8)
        cc1a_out = dram.tile([n_cores, KH], FP8, addr_space="Shared")
        cc1b_in = dram.tile([KH], FP8)
        cc1b_out = dram.tile([n_cores, KH], FP8, addr_space="Shared")
        VH2 = TLH * P * D2     # half of a Vc0 d-half (tile pair)
        cc2a_in = dram.tile([VNB + VH2 + XRB], U8)
        cc2a_out = dram.tile([n_cores, VNB + VH2 + XRB], U8, addr_space="Shared")
        cc2b_in = dram.tile([VH2], U8)
        cc2b_out = dram.tile([n_cores, VH2], U8, addr_space="Shared")

        consts = top.enter_context(tc.tile_pool(name="consts", bufs=1))
        iu_sb = consts.tile([P, 2 * P], BF16)
        nc.scalar.dma_start(iu_sb[:], c_iu.ap())
        stril_sb = consts.tile([T, T], BF16)
        nc.scalar.dma_start(stril_sb[:], c_stril.ap())
        selp_sb = consts.tile([P, NPAIR * 2 * MB], FP8)
        nc.scalar.dma_start(selp_sb[:], c_selp.ap())
        ident_sb = consts.tile([P, P], F32)
        nc.scalar.dma_start(ident_sb[:], c_ident.ap())
        expb_sb = consts.tile([P, 1], F32)
        nc.vector.memset(expb_sb[:], EXPB)

        persist = top.enter_context(tc.tile_pool(name="persist", bufs=1))
        q2T = persist.tile([P, DC * B], FP8)         # (q @ wk.T).T row block
        st = persist.tile([P, T * B], FP8)           # exp(scores)/16, transposed
        vs_sb = persist.tile([T, D], BF16)           # per-tile V colsums
        xrs_s = persist.tile([P, n_cores * DC * TL], BF16)  # gathered x row sums [p,(c dc t)]
        xrs_g = persist.tile([P, DC * T], BF16)             # re-strided to [p,(dc r)]
        suf_sb = persist.tile([T, B], BF16)
        cs_sb = persist.tile([T, B], BF16)
        recip = persist.tile([P, TL], F32)
        dennat = persist.tile([P, TL], F32)
        den_pad = persist.tile([P, B], F32)
        wv_sb = persist.tile([P, DC * D], BF16)
        vpg = [[persist.tile([P, TL * D2], FP8, name=f"vp{g}_{rc}")
                for rc in range(n_cores)] for g in range(2)]

        # ------------- stage 1: transposes, gather-x, projections -------------
        with ExitStack() as s1:
            xpool = s1.enter_context(tc.tile_pool(name="xload", bufs=6))
            xTp = s1.enter_context(tc.tile_pool(name="xT", bufs=1))
            xT8 = xTp.tile([P, DC * B], FP8)     # x.T   (scores lhsT + cc1 input)
            xcT = xTp.tile([P, DC * B], BF16)    # prefix-x.T (V path)
            xrs_f = xTp.tile([P, DC * TL], F32)
            xrs_bf = xTp.tile([P, DC * TL], BF16)

            wpool = s1.enter_context(tc.tile_pool(name="w", bufs=1))
            wq_sb = wpool.tile([P, DC * D], FP8)
            wkt_sb = wpool.tile([P, DC * D], FP8)
            qT = wpool.tile([P, DC * B], FP8)

            trps = s1.enter_context(tc.tile_pool(name="trps", bufs=2, space="PSUM"))
            # x.T first (I pass) -> cc1 trigger ASAP; prefix-x.T (U pass) after
            xts = []
            for tcc in range(TL):
                xt_ = xpool.tile([P, D], BF16, tag="x", name=f"xt_{tcc}")
                (nc.sync if tcc < 2 else nc.scalar).dma_start(
                    xt_[:], x.ap()[tcc * P:(tcc + 1) * P, :])
                xts.append(xt_)
            cc1av = cc1a_in[0:KH].rearrange("(p k) -> p k", p=P)
            cc1bv = cc1b_in[0:KH].rearrange("(p k) -> p k", p=P)
            for dc in range(DC):
                psI = trps.tile([P, B], F32, tag="trI")
                for tcc in range(TL):
                    nc.tensor.matmul(psI[:, tcc * P:(tcc + 1) * P],
                                     xts[tcc][:, dc * P:(dc + 1) * P], iu_sb[:, 0:P],
                                     start=True, stop=True)
                (nc.vector.tensor_copy if dc % 2 == 0 else nc.scalar.copy)(
                    xT8[:, dc * B:(dc + 1) * B], psI[:])
                # stream x.T key-halves to the collective inputs as they land
                nc.sync.dma_start(cc1av[:, dc * B2:(dc + 1) * B2],
                                  xT8[:, dc * B:dc * B + B2])
                nc.sync.dma_start(cc1bv[:, dc * B2:(dc + 1) * B2],
                                  xT8[:, dc * B + B2:(dc + 1) * B])

            # gather x.T immediately (GpSimd queue is otherwise empty)
            nc.gpsimd.collective_compute(
                "AllGather", ALU.bypass,
                replica_groups=[list(range(n_cores))],
                ins=[cc1a_in.opt()], outs=[cc1a_out.opt()],
            )
            nc.gpsimd.collective_compute(
                "AllGather", ALU.bypass,
                replica_groups=[list(range(n_cores))],
                ins=[cc1b_in.opt()], outs=[cc1b_out.opt()],
            )

            for dc in range(DC):
                psU = trps.tile([P, B], F32, tag="trU")
                for tcc in range(TL):
                    nc.tensor.matmul(psU[:, tcc * P:(tcc + 1) * P],
                                     xts[tcc][:, dc * P:(dc + 1) * P], iu_sb[:, P:2 * P],
                                     start=True, stop=True)
                (nc.scalar.copy if dc % 2 == 0 else nc.vector.tensor_copy)(
                    xcT[:, dc * B:(dc + 1) * B], psU[:])
                nc.vector.tensor_copy(
                    xrs_f[:, dc * TL:(dc + 1) * TL]
                    .rearrange("p (t one) -> p t one", one=1),
                    psU.rearrange("p (t j) -> p t j", j=P)[:, :, P - 1:P])

            # weights (HWDGE queues; nothing gates the collective).
            # wv first: the V path gates cc2, which gates phase B.
            for dc in range(DC):
                nc.scalar.dma_start(wv_sb[:, dc * D:(dc + 1) * D], wv_d.ap()[dc * P:(dc + 1) * P, :])
            for dc in range(DC):
                nc.sync.dma_start(wq_sb[:, dc * D:(dc + 1) * D], wq_d.ap()[dc * P:(dc + 1) * P, :])
            for dc in range(DC):
                nc.scalar.dma_start(wkt_sb[:, dc * D:(dc + 1) * D], wkt_d.ap()[dc * P:(dc + 1) * P, :])

            pps = s1.enter_context(tc.tile_pool(name="pps", bufs=4, space="PSUM"))

            # Vc0 row blocks (within-tile prefix V, from prefix-x.T) -> cc2.
            # cc2a = d-half-0 (all tiles) + d-half-1 tiles {0,1} + xrs; cc2b =
            # d-half-1 tiles {2,3} — matches phase B's consumption order.
            cc2av0 = cc2a_in[0:VNB].rearrange("(p k) -> p k", p=P)
            cc2av1 = cc2a_in[VNB:VNB + VH2].rearrange("(p k) -> p k", p=P)
            cc2bv = cc2b_in[0:VH2].rearrange("(p k) -> p k", p=P)
            for nh in range(NH):
                for tcc in range(TL):
                    vl = xpool.tile([P, D2], FP8, tag="vl")
                    v_ps = pps.tile([P, W], F32, tag="pp", name="v_ps")
                    for dci in range(DC):
                        nc.tensor.matmul(
                            v_ps[:],
                            xcT[:, dci * B + tcc * P: dci * B + (tcc + 1) * P],
                            wv_sb[:, dci * D + nh * W: dci * D + (nh + 1) * W],
                            start=(dci == 0), stop=(dci == DC - 1),
                        )
                    (nc.vector.tensor_copy if tcc % 2 == 0 else nc.scalar.copy)(
                        vl[:], v_ps[:])
                    if nh == 0:
                        dst = cc2av0[:, tcc * D2:(tcc + 1) * D2]
                    elif tcc < TLH:
                        dst = cc2av1[:, tcc * D2:(tcc + 1) * D2]
                    else:
                        dst = cc2bv[:, (tcc - TLH) * D2:(tcc - TLH + 1) * D2]
                    nc.gpsimd.dma_start(dst, vl[:].bitcast(U8))
                if nh == 0:
                    # per-tile x row sums (f32 exact, from the U-part last col)
                    nc.vector.tensor_copy(xrs_bf[:], xrs_f[:])
                    cc2x = cc2a_in[VNB + VH2:VNB + VH2 + XRB].rearrange("(p k) -> p k", p=P)
                    nc.gpsimd.dma_start(cc2x[:, :], xrs_bf[:].bitcast(U8))

            # qT = (x @ wq).T  then  q2T = (q @ wk.T).T   (both fp8 DoubleRow)
            wq3 = wq_sb.rearrange("p (dc d) -> p dc d", dc=DC)
            wkt3 = wkt_sb.rearrange("p (dc d) -> p dc d", dc=DC)
            xT83 = xT8.rearrange("p (dc b) -> p dc b", dc=DC)
            for dco in range(DC):
                q_ps = pps.tile([P, B], F32, tag="pp", name="q_ps")
                for pp_ in range(DC // 2):
                    nc.tensor.matmul(
                        q_ps[:],
                        wq3[:, 2 * pp_:2 * pp_ + 2, dco * P:(dco + 1) * P],
                        xT83[:, 2 * pp_:2 * pp_ + 2, :],
                        start=(pp_ == 0), stop=(pp_ == DC // 2 - 1),
                        perf_mode=mybir.MatmulPerfMode.DoubleRow,
                    )
                nc.vector.tensor_copy(qT[:, dco * B:(dco + 1) * B], q_ps[:])
            qT3 = qT.rearrange("p (dc b) -> p dc b", dc=DC)
            for dco in range(DC):
                q2_ps = pps.tile([P, B], F32, tag="pp", name="q2_ps")
                for pp_ in range(DC // 2):
                    nc.tensor.matmul(
                        q2_ps[:],
                        wkt3[:, 2 * pp_:2 * pp_ + 2, dco * P:(dco + 1) * P],
                        qT3[:, 2 * pp_:2 * pp_ + 2, :],
                        start=(pp_ == 0), stop=(pp_ == DC // 2 - 1),
                        perf_mode=mybir.MatmulPerfMode.DoubleRow,
                    )
                nc.vector.tensor_copy(q2T[:, dco * B:(dco + 1) * B], q2_ps[:])

        # second collective pair: gather Vc0 halves (+xrs with the first)
        nc.gpsimd.collective_compute(
            "AllGather", ALU.bypass,
            replica_groups=[list(range(n_cores))],
            ins=[cc2a_in.opt()], outs=[cc2a_out.opt()],
        )
        nc.gpsimd.collective_compute(
            "AllGather", ALU.bypass,
            replica_groups=[list(range(n_cores))],
            ins=[cc2b_in.opt()], outs=[cc2b_out.opt()],
        )
        # gathered x row sums: one DMA, 3D AP over source cores
        nc.sync.dma_start(
            xrs_s[:].bitcast(U8).rearrange("p (c k) -> p c k", c=n_cores),
            cc2a_out[0:n_cores, VNB + VH2:VNB + VH2 + XRB]
            .rearrange("c (p k) -> p c k", p=P))
        # prefetch gathered Vc0 blocks (fires as soon as each gather lands)
        for rc in range(n_cores):
            nc.gpsimd.dma_start(
                vpg[0][rc][:].bitcast(U8),
                cc2a_out[rc, 0:VNB].rearrange("(p k) -> p k", p=P))
        for rc in range(n_cores):
            nc.gpsimd.dma_start(
                vpg[1][rc][:, 0:TLH * D2].bitcast(U8),
                cc2a_out[rc, VNB:VNB + VH2].rearrange("(p k) -> p k", p=P))

        # ------------------- phase A: scores / exp / CS+den -------------------
        with ExitStack() as pa:
            ktp = pa.enter_context(tc.tile_pool(name="kt", bufs=3))
            ztp = pa.enter_context(tc.tile_pool(name="zt", bufs=3, space="PSUM"))
            csp = pa.enter_context(tc.tile_pool(name="csp", bufs=1, space="PSUM"))
            sfp = pa.enter_context(tc.tile_pool(name="sfp", bufs=2, space="PSUM"))
            cs_ps = csp.tile([33, B], F32)

            q2T3 = q2T.rearrange("p (dc b) -> p dc b", dc=DC)
            pi = 0
            for h in range(2):
                cch = cc1a_out if h == 0 else cc1b_out
                for rc in range(n_cores):
                    ktc = ktp.tile([P, DC * B2], FP8, tag="kt")
                    nc.sync.dma_start(ktc[:], cch[rc, 0:KH].rearrange("(p k) -> p k", p=P))
                    ktc3 = ktc.rearrange("p (dc i) -> p dc i", dc=DC)
                    for sub2 in range(TLH):
                        rg = rc * TL + h * TLH + sub2
                        zt = ztp.tile([P, B], F32, tag="zt")
                        for pp in range(DC // 2):
                            nc.tensor.matmul(
                                zt[:],
                                ktc3[:, 2 * pp:2 * pp + 2, sub2 * P:(sub2 + 1) * P],
                                q2T3[:, 2 * pp:2 * pp + 2, :],
                                start=(pp == 0), stop=(pp == DC // 2 - 1),
                                perf_mode=mybir.MatmulPerfMode.DoubleRow,
                            )
                        nc.scalar.activation(st[:, rg * B:(rg + 1) * B], zt[:],
                                             AF.Exp, bias=expb_sb[:], scale=scale)
                        if sub2 % 2 == 1:
                            pr = rg // 2
                            lp = (selp_sb[:, pr * 2 * MB:(pr + 1) * 2 * MB]
                                  .rearrange("p (two m) -> p two m", two=2)[:, :, 0:33])
                            rp = (st[:, (rg - 1) * B:(rg + 1) * B]
                                  .rearrange("p (two b) -> p two b", two=2))
                            nc.tensor.matmul(
                                cs_ps[:], lp, rp,
                                start=(pi == 0), stop=(pi == NPAIR - 1),
                                perf_mode=mybir.MatmulPerfMode.DoubleRow,
                            )
                            pi += 1

            for rc in range(n_cores):
                nc.scalar.dma_start(
                    vpg[1][rc][:, TLH * D2:TL * D2].bitcast(U8),
                    cc2b_out[rc, 0:VH2].rearrange("(p k) -> p k", p=P))

            nc.vector.tensor_copy(cs_sb[:], cs_ps[0:T, :])
            nc.vector.memset(den_pad[:], 0.0)
            nc.vector.tensor_copy(den_pad[32:33, :], cs_ps[32:33, :])
            suf_ps = sfp.tile([T, B], F32)
            nc.tensor.matmul(suf_ps[:], stril_sb[:], cs_sb[:], start=True, stop=True)
            nc.scalar.copy(suf_sb[:], suf_ps[:])

            # 0.03125/den now, so the phase-B epilogues are never gated on it
            for sub in range(TL):
                dps = sfp.tile([P, P], F32, tag="dtp")
                nc.tensor.transpose(dps[:], den_pad[:, sub * P:(sub + 1) * P], ident_sb[:])
                nc.vector.tensor_scalar(dennat[:, sub:sub + 1], dps[:, 32:33], 32.0,
                                        None, op0=ALU.mult)
            nc.vector.reciprocal(recip[:], dennat[:])

        # ------------- phase B: num accumulation (natural layout) -------------
        # num[i, d] = sum_t st[t, i] Vc0[t, d] + sum_r SUF[r, i] VS[r, d];
        # st tile pairs are the stationary operand, Vc0 pairs the moving one,
        # so the output lands directly in [query, feature] layout: no
        # transposes, and the den scale is a per-partition scalar multiply.
        # VS is computed between the g0 sweep and its closes so the PE never
        # stalls on the xrs gather at the phase boundary.
        with ExitStack() as pb:
            outp = pb.enter_context(tc.tile_pool(name="outp", bufs=4))
            nump = pb.enter_context(tc.tile_pool(name="nump", bufs=4, space="PSUM"))
            vsps = pb.enter_context(tc.tile_pool(name="vsps", bufs=2, space="PSUM"))

            def sweep(g, nums):
                # pair-major: tiles {0,1} of every core first, so g1's second
                # half (the last collective's payload) is needed latest
                for pr in range(TL // 2):
                    for rc in range(n_cores):
                        vp3 = vpg[g][rc].rearrange("p (t d) -> p t d", t=TL)
                        rgb = (rc * TL + 2 * pr) * B
                        stp3 = (st[:, rgb: rgb + 2 * B]
                                .rearrange("p (two b) -> p two b", two=2))
                        rhs = vp3[:, 2 * pr:2 * pr + 2, :]
                        for ic in range(TL):
                            nc.tensor.matmul(
                                nums[ic][:],
                                stp3[:, :, ic * P:(ic + 1) * P],
                                rhs,
                                start=(rc == 0 and pr == 0), stop=False,
                                perf_mode=mybir.MatmulPerfMode.DoubleRow,
                            )

            def close_group(g, nums):
                for ic in range(TL):
                    nc.tensor.matmul(
                        nums[ic][:], suf_sb[:, ic * P:(ic + 1) * P],
                        vs_sb[:, g * D2:(g + 1) * D2],
                        start=False, stop=True,
                    )
                    ot = outp.tile([P, D2], F32, tag="ot", name=f"ot{g}_{ic}")
                    if ic % 2 == 0:
                        nc.vector.tensor_scalar(ot[:], nums[ic][:],
                                                recip[:, ic:ic + 1], None, op0=ALU.mult)
                    else:
                        nc.scalar.activation(ot[:], nums[ic][:], AF.Copy,
                                             scale=recip[:, ic:ic + 1])
                    (nc.sync if ic % 2 == 0 else nc.scalar).dma_start(
                        out.ap()[ic * P:(ic + 1) * P, g * D2:(g + 1) * D2], ot[:])

            nums0 = [nump.tile([P, D2], F32, tag="num", name=f"num_ps0_{ic}")
                     for ic in range(TL)]
            sweep(0, nums0)

            # VS = xrs.T-mm(wv)  [T, D] (gathered x row sums, re-strided)
            nc.vector.tensor_copy(
                xrs_g.rearrange("p (dc c t) -> p dc c t", dc=DC, c=n_cores),
                xrs_s.rearrange("p (c dc t) -> p dc c t", c=n_cores, dc=DC))
            for nh in range(NH):
                vs_ps = vsps.tile([T, W], F32, tag="vs")
                for dci in range(DC):
                    nc.tensor.matmul(
                        vs_ps[:],
                        xrs_g[:, dci * T:(dci + 1) * T],
                        wv_sb[:, dci * D + nh * W: dci * D + (nh + 1) * W],
                        start=(dci == 0), stop=(dci == DC - 1),
                    )
                nc.vector.tensor_copy(vs_sb[:, nh * W:(nh + 1) * W], vs_ps[:])

            close_group(0, nums0)
            nums1 = [nump.tile([P, D2], F32, tag="num", name=f"num_ps1_{ic}")
                     for ic in range(TL)]
            sweep(1, nums1)
            close_group(1, nums1)

    nc.compile()
    return nc


def make_in_maps(x_full, wq, wk, wv, n_cores=8):
    import ml_dtypes
    bf = lambda a: np.ascontiguousarray(a).astype(ml_dtypes.bfloat16)
    f8 = lambda a: np.ascontiguousarray(a).astype(ml_dtypes.float8_e4m3)
    SEQ, D = x_full.shape
    T = SEQ // P
    B = SEQ // n_cores
    consts = make_consts(T)
    wq8 = f8(wq * 8.0)
    wkt2 = f8(wk.T * 2.0)
    wvb = bf(wv)
    in_maps = []
    for c in range(n_cores):
        m = {"x": bf(x_full[c * B:(c + 1) * B]),
             "wq": wq8, "wk": wkt2, "wv": wvb}
        m.update(consts)
        in_maps.append(m)
    return in_maps


def algo_ref(x, wq, wk, wv):
    """Numpy float64 reference of the restructured math (for validation)."""
    x = x.astype(np.float64)
    q2 = (x @ wq.astype(np.float64)) @ wk.astype(np.float64).T
    s = np.exp(q2 @ x.T / np.sqrt(x.shape[1]))
    Vc = np.cumsum(x @ wv.astype(np.float64), axis=0)
    num = s @ Vc
    den = s @ (np.arange(x.shape[0]) + 1.0)
    return (num / den[:, None]).astype(np.float32)


# ----------------------------------------------------------------------------
# Harness entry point: full (unsharded) inputs -> full output.
# ----------------------------------------------------------------------------
SEQ, D_IN, N_CORES = 4096, 1024, 8
_built = {}


def _get_nc(SEQ_=SEQ, D_=D_IN, n_cores=N_CORES):
    key = (SEQ_, D_, n_cores)
    if key not in _built:
        _built[key] = build(SEQ=SEQ_, D=D_, n_cores=n_cores)
    return _built[key]


def run(x, wq, wk, wv, trace=False, **spmd_kwargs):
    from concourse.bass_utils import run_bass_kernel_spmd

    x = np.ascontiguousarray(np.asarray(x, dtype=np.float32))
    wq = np.ascontiguousarray(np.asarray(wq, dtype=np.float32))
    wk = np.ascontiguousarray(np.asarray(wk, dtype=np.float32))
    wv = np.ascontiguousarray(np.asarray(wv, dtype=np.float32))
    n_cores = N_CORES
    nc = _get_nc(x.shape[0], x.shape[1], n_cores)
    in_maps = make_in_maps(x, wq, wk, wv, n_cores=n_cores)
    res = run_bass_kernel_spmd(nc, in_maps, list(range(n_cores)),
                               trace=trace, **spmd_kwargs)
    out = np.concatenate([res.results[c]["out"] for c in range(n_cores)], axis=0)
    return out, res


def kernel(x, wq, wk, wv):
    out, _ = run(x, wq, wk, wv, trace=False)
    return out
